# revision 1
# baseline (speedup 1.0000x reference)
"""DGINConv (2-layer GIN with edge features) Trainium2 kernel — sparse/packed.

Math (per layer, reference):
    hb[j,:] = Wnode @ x[j] + bne                       # [N, D] node term
    he[i,j,:] = We @ edges[i,j,:]                      # edge term
    msg[i,:] = sum_{j: adj[i,j]=1} relu(hb[j,:] + he[i,j,:])
    out = relu(Wn @ ((1+eps)*x[i] + msg[i]) + bn)

adj density is ~3%, so instead of the dense [128 own-rows x 1024 j] sweep we
pack each own row's ~31 neighbors into padded slots (host-side, from the
actual adj at runtime):
  - own rows sorted by degree (host permutation), grouped into 8 chunks of
    16 rows; chunk c padded to S_c slots/row (mult of 4).  Q = 16*sum(S_c).
  - peTA [34, Q] bf16: packed edge vectors ++ a bias-carrier ones row ++ a
    -1e9 pad-mask row (so pad slots go negative and relu kills them)
  - xgT [128, Q] bf16: host-pregathered source-node features per slot
  - slot -> source-node index lists (int16) for the on-device gather; PAD
    slots point at column 1024 of hbT which holds -1e9.

Layer 0 (x known on host): per chunk PSUM = [We;bne;1]@peTA + Wnode@xgT
(two PE matmuls, K=34 + K=128), exit = plain ACT relu -> r bf16.

Layer 1 (x = allgathered h1, device-only): hbT = Wnode@x + bne (PE + DVE),
hbg[d,q] = ap_gather(hbT, idx) on POOL (custom ucode op), psum = We@peTA
(PE, K=32); exits: DVE custom relu(in0+in1) reading PSUM+hbg, or POOL
bf16-convert + PE identity-inject + ACT relu (split tuned via L1_EXIT).

Fold (both layers): msg[d,i] = segment-sum over S_c slots = bf16 2x
pairwise tree (DVE) + tensor_reduce;  h = relu(Wn @ ((1+eps)x + msg) + bn)
computed in two 64-column halves to overlap the output path.

Between layers: transpose h1 -> [i,d], AllGather (rows stay in per-core
sorted order; layer-2 gather indices are host-remapped to that layout).
Final output rows are un-permuted on the host.

Distribution: destination rows sharded 8 ways; nodes/weights replicated;
one AllGather of updated node features between layers.
"""

import sys

if "/opt/trn_rl_repo" not in sys.path:
    sys.path.insert(0, "/opt/trn_rl_repo")

import numpy as np

N, D, E, NC = 1024, 128, 32, 8
SH = N // NC          # 128 rows per core
NCH = 8               # chunks of sorted own-rows
CHI = SH // NCH       # 16 rows per chunk
PAD = N               # hbT column holding -1e9
HBW = N + 8           # hbT width (pad cols 1024..1032)

# Chunk slot counts (padded max degree per 16-row chunk of the degree-sorted
# rows, mult of 4, same for all cores).  Recomputed from the actual adj at
# runtime; this is the value for the reference setup_inputs() graph.
S_DEFAULT = (52, 40, 36, 36, 32, 32, 28, 28)

# engine assignment knobs (tuned against TimelineSim)
L0_EXIT = "AAAAAAHH"   # 'A' ACT relu | 'D' DVE relu | 'H' ACT+DVE halves
L1_EXIT = "BBBBBBBB"   # 'D' DVE custom | 'A'/'P' convert+inject | 'B' bitcast-inject
TREE0 = "DDDDDDDD"     # 'D' DVE | 'P' POOL (L1+L2 tree levels)
TREE1 = "DDDDDDDD"
GATHER_SPLIT = 4       # ap_gather instructions per layer
HB_SPLIT = False       # split hb-exit across DVE+ACT
HB_ENG = "D"           # hb-exit engine when not split: 'D' DVE | 'A' ACT
CONV_PREP = False      # emit l1 converts inside hb_prep (early pool/ACT order)
IDX_EARLY = False      # idx12 DMA before peTA pieces
CORDER = "N"           # layer1 chunk emission order: 'N' natural | 'D' D-first
XG_FP8 = True          # xgT in fp8e4m3 (halves the biggest input DMA)
FIN_ENG = "A"          # final relu+bias: 'A' ACT activation | 'D' DVE

_cache = {}
_CUSTOM = {}


def _ensure_custom_op():
    """Register RELU_ADD_REDUCE_GIN: out = relu(in0 + in1); accum = sum."""
    if "op" in _CUSTOM:
        return _CUSTOM["op"]
    import concourse.dve_ops as dve_ops
    from concourse.dve_spec import Spec, Src0, Src1, relu, lower, _has_src1
    from concourse.dve_spec import Zero
    from concourse.dve_uop import DveOpSpec
    from operator import add

    name = "RELU_ADD_REDUCE_GIN"

    def _ref(in0, in1, c0, c1, c2):
        b = dve_ops._dve_relu(in0.astype(np.float32) + in1.astype(np.float32))
        return b, b.reshape(b.shape[0], -1).sum(axis=-1, keepdims=True)

    spec = Spec(body=relu(Src0 + Src1), accum=add, accum_init=Zero,
                reference=_ref)
    row = dve_ops._CUSTOM_DVE_ROW_BASE + len(dve_ops.OPS)
    assert row < 0x20
    shas = {}
    for ver in ("v3", "v4"):
        try:
            s = DveOpSpec(name=name, opcode=row, uops=lower(spec, ver=ver),
                          rd1_en=_has_src1(spec))
            shas[ver] = s.sha(ver)
        except Exception:
            pass
    op = dve_ops.DveOp(name, spec, subdim=False, uops_sha=shas)
    dve_ops.OPS.append(op)
    dve_ops.CUSTOM_DVE_SPECS[name] = spec
    dve_ops._SUB_OPCODE_FOR_NAME[name] = row
    _CUSTOM["op"] = op
    return op


def _build_nc(mode="full", S=S_DEFAULT):
    from contextlib import ExitStack

    import concourse.mybir as mybir
    import concourse.tile as tile
    from concourse import bacc

    f32 = mybir.dt.float32
    bf16 = mybir.dt.bfloat16
    i16 = mybir.dt.int16
    RELU = mybir.ActivationFunctionType.Relu
    IDENT = mybir.ActivationFunctionType.Identity
    ADD = mybir.AluOpType.add
    MAX = mybir.AluOpType.max
    MULT = mybir.AluOpType.mult

    relu_add = _ensure_custom_op()
    S = tuple(S)
    Q = CHI * sum(S)
    cbase = [CHI * sum(S[:c]) for c in range(NCH)]

    nc = bacc.Bacc("TRN2", target_bir_lowering=False, debug=False,
                   enable_asserts=False, num_devices=NC)

    def din(name, shape, dt=None):
        return nc.dram_tensor(name, shape, dt or f32, kind="ExternalInput").ap()

    peTA_d = din("peTA_sh", [34, Q], bf16)   # edges^T ++ ones ++ padmask rows
    fp8 = mybir.dt.float8e4
    xg_dt = fp8 if XG_FP8 else bf16
    xgT_d = din("xgT_sh", [D, Q], xg_dt)     # host-gathered nodes^T (pads 0)
    idx_d = din("idx_sh", [128, 2 * (Q // 16)], i16)
    xT_d = din("xT", [D, N + SH], bf16)   # nodes.T ++ own-sorted.T
    Wb_d = din("Wb", [D, 5 * D], bf16)   # WnodeT0|WnT0|WnodeT1|WnT1|I
    WeA_d = din("WeA", [34, 2 * D], bf16)    # [WeT0;bne0;ones] | [WeT1;--]
    bias_d = din("bias", [D, 5])             # bne0|bn0|bne1|bn1|opse
    out_d = nc.dram_tensor("out", [SH, D], f32, kind="ExternalOutput").ap()

    with tile.TileContext(nc) as tc, ExitStack() as ctx:
        P = ctx.enter_context(tc.tile_pool(name="persist", bufs=1))
        dramp = ctx.enter_context(tc.tile_pool(name="dram", bufs=1, space="DRAM"))
        psumC = ctx.enter_context(tc.tile_pool(name="psumC", bufs=3, space="PSUM"))
        psumF = ctx.enter_context(tc.tile_pool(name="psumF", bufs=2, space="PSUM"))
        scrp = ctx.enter_context(tc.tile_pool(name="scr", bufs=3))

        # ---------------- inputs (2 queues, priority order) -------------
        # DMA_ENGINES serialize transfers in the cost model, so order and
        # granularity matter: critical small tensors first, big packed
        # tensors in chunk-pair pieces so compute starts early.
        xTA = P.tile([D, N + SH], bf16)
        nc.sync.dma_start(out=xTA[:], in_=xT_d[:])
        Wb = P.tile([D, 5 * D], bf16)
        nc.scalar.dma_start(out=Wb[:], in_=Wb_d[:])
        WeA = P.tile([34, 2 * D], bf16)
        nc.scalar.dma_start(out=WeA[:], in_=WeA_d[:])
        peTA = P.tile([34, Q], bf16)
        xgT = P.tile([D, Q], xg_dt)
        idx12 = P.tile([128, 2 * (Q // 16)], i16)
        # chunk-0 operands race on the SP queue so layer-0 starts early
        nc.sync.dma_start(out=xgT[:, 0:cbase[2]], in_=xgT_d[:, 0:cbase[2]])
        bias = P.tile([D, 5], f32)
        nc.scalar.dma_start(out=bias[:], in_=bias_d[:])
        nc.sync.dma_start(out=peTA[:, 0:cbase[2]], in_=peTA_d[:, 0:cbase[2]])
        nc.sync.dma_start(out=idx12[:], in_=idx_d[:])
        bounds = [cbase[2]] + [cbase[c] for c in range(4, NCH, 2)] + [Q]
        for i in range(len(bounds) - 1):
            lo, hi = bounds[i], bounds[i + 1]
            nc.sync.dma_start(out=peTA[:, lo:hi], in_=peTA_d[:, lo:hi])
            nc.scalar.dma_start(out=xgT[:, lo:hi], in_=xgT_d[:, lo:hi])


        ident = Wb[:, 4 * D:5 * D]
        hbT = P.tile([D, HBW], f32)
        nc.gpsimd.memset(hbT[:, N:HBW], -1e9)
        dve_scrap = P.tile([128, 1], f32)

        def fold(r, c, Sc, msg, tree_eng):
            """r [128, CHI, Sc] bf16 -> msg[:, chunk c] via L1 tree + reduce."""
            teng = nc.gpsimd if tree_eng == "P" else nc.vector
            h1 = Sc // 2
            t1 = scrp.tile([128, CHI, h1], bf16, tag=f"t1{Sc}")
            teng.tensor_tensor(out=t1[:], in0=r[:, :, 0:h1],
                               in1=r[:, :, h1:Sc], op=ADD)
            if h1 % 2 == 0:
                h2 = h1 // 2
                t2 = scrp.tile([128, CHI, h2], bf16, tag=f"t2{Sc}")
                teng.tensor_tensor(out=t2[:], in0=t1[:, :, 0:h2],
                                   in1=t1[:, :, h2:h1], op=ADD)
            else:
                t2 = t1
            nc.vector.tensor_reduce(
                out=msg[:, CHI * c:CHI * (c + 1)], in_=t2[:],
                axis=mybir.AxisListType.X, op=ADD)

        def finish(l, msg, xsT_l):
            WnT = Wb[:, 2 * D * l + D:2 * D * l + 2 * D]
            bn = bias[:, 2 * l + 1:2 * l + 2]
            hT = P.tile([D, SH], bf16, tag=f"hT{l}")
            for hh in range(2):
                sl = slice(64 * hh, 64 * (hh + 1))
                xt, xo = xsT_l
                z_bf = P.tile([D, 64], bf16, tag=f"zbf{l}{hh}")
                nc.vector.scalar_tensor_tensor(
                    out=z_bf[:], in0=xt[:, xo + 64 * hh:xo + 64 * (hh + 1)],
                    scalar=bias[:, 4:5], in1=msg[:, sl], op0=MULT, op1=ADD)
                ps_h = psumF.tile([D, 64], f32, tag="fin")
                nc.tensor.matmul(out=ps_h[:], lhsT=WnT, rhs=z_bf[:],
                                 start=True, stop=True)
                if FIN_ENG == "D":
                    nc.vector.tensor_scalar(out=hT[:, sl], in0=ps_h[:],
                                            scalar1=bn, scalar2=0.0,
                                            op0=ADD, op1=MAX)
                else:
                    nc.scalar.activation(out=hT[:, sl], in_=ps_h[:],
                                         func=RELU, bias=bn)
            return hT

        # ---------------- layer 0: host-pregathered node term ----------
        def layer0(xsT_l):
            WnodeT = Wb[:, 0:D]
            msg = P.tile([D, SH], f32, tag="msg0")
            for c in range(NCH):
                W = CHI * S[c]
                ps = psumC.tile([128, W], f32, tag="chunk")
                for s0 in range(0, W, 512):
                    s1 = min(s0 + 512, W)
                    nc.tensor.matmul(out=ps[:, s0:s1], lhsT=WeA[:, 0:D],
                                     rhs=peTA[:, cbase[c] + s0:cbase[c] + s1],
                                     start=True, stop=False)
                    nc.tensor.matmul(out=ps[:, s0:s1], lhsT=WnodeT,
                                     rhs=xgT[:, cbase[c] + s0:cbase[c] + s1],
                                     start=False, stop=True)
                r = scrp.tile([128, CHI, S[c]], bf16, tag=f"r{S[c]}")
                r2 = r[:].rearrange("p a b -> p (a b)")
                if L0_EXIT[c] == "A":
                    nc.scalar.activation(out=r2, in_=ps[:], func=RELU)
                elif L0_EXIT[c] == "H":
                    h = W // 2
                    r2a = r[:].rearrange("p a b -> p (a b)")
                    nc.scalar.activation(out=r2a[:, 0:h], in_=ps[:, 0:h],
                                         func=RELU)
                    nc.vector.tensor_scalar(out=r2a[:, h:W], in0=ps[:, h:W],
                                            scalar1=0.0, scalar2=None, op0=MAX)
                else:
                    nc.vector.tensor_scalar(out=r2, in0=ps[:], scalar1=0.0,
                                            scalar2=None, op0=MAX)
                fold(r, c, S[c], msg, TREE0[c])
            return finish(0, msg, xsT_l)

        # ------------- layer 1: on-device hb + POOL gather --------------
        def hb_prep(l, xT_l, idx_half):
            WnodeT = Wb[:, 2 * D * l:2 * D * l + D]
            bne = bias[:, 2 * l:2 * l + 1]
            iof = idx_half * (Q // 16)
            psH = psumC.tile([D, N], f32, tag="chunk")
            for h in range(2):
                nc.tensor.matmul(out=psH[:, 512 * h:512 * (h + 1)],
                                 lhsT=WnodeT, rhs=xT_l(h),
                                 start=True, stop=True)
            if HB_ENG == "D2":
                for h in range(2):
                    nc.vector.tensor_scalar(
                        out=hbT[:, 512 * h:512 * (h + 1)],
                        in0=psH[:, 512 * h:512 * (h + 1)],
                        scalar1=bne, scalar2=None, op0=ADD)
            elif HB_SPLIT:
                nc.vector.tensor_scalar(out=hbT[:, 0:512], in0=psH[:, 0:512],
                                        scalar1=bne, scalar2=None, op0=ADD)
                nc.scalar.activation(out=hbT[:, 512:N], in_=psH[:, 512:N],
                                     func=IDENT, bias=bne)
            elif HB_ENG == "D":
                nc.vector.tensor_scalar(out=hbT[:, 0:N], in0=psH[:],
                                        scalar1=bne, scalar2=None, op0=ADD)
            else:
                nc.scalar.activation(out=hbT[:, 0:N], in_=psH[:],
                                     func=IDENT, bias=bne)
            hbg = P.tile([D, Q], f32, tag=f"hbg{l}")
            per = (NCH + GATHER_SPLIT - 1) // GATHER_SPLIT
            for g in range(0, NCH, per):
                lo = cbase[g]
                hi = cbase[g + per] if g + per < NCH else Q
                nc.gpsimd.ap_gather(
                    out_ap=hbg[:, lo:hi], in_ap=hbT[:],
                    idxs_ap=idx12[:, iof + lo // 16:iof + hi // 16],
                    channels=128, num_elems=HBW, d=1, num_idxs=hi - lo)
            hgbs = {}

            def conv(c):
                W = CHI * S[c]
                hgb = scrp.tile([128, W], bf16, tag=f"hgb{S[c]}")
                if L1_EXIT[c] == "P":
                    nc.gpsimd.tensor_scalar(
                        out=hgb[:], in0=hbg[:, cbase[c]:cbase[c] + W],
                        scalar1=0.0, scalar2=None, op0=ADD)
                else:
                    nc.scalar.activation(
                        out=hgb[:], in_=hbg[:, cbase[c]:cbase[c] + W],
                        func=IDENT)
                hgbs[c] = hgb

            if CONV_PREP:
                for c in range(NCH):
                    if L1_EXIT[c] in "AP":
                        conv(c)
            return hbg, hgbs, conv

        def layer1(l, hbg, hgbs, conv, xsT_l):
            hbg_bf = hbg[:].bitcast(bf16).rearrange(
                "p (q two) -> p q two", two=2)
            WeT = WeA[0:32, D:2 * D]
            msg = P.tile([D, SH], f32, tag=f"msg{l}")
            corder = list(range(NCH))
            if CORDER == "D":
                corder.sort(key=lambda c: 0 if L1_EXIT[c] == "D" else 1)
            for c in corder:
                W = CHI * S[c]
                act = L1_EXIT[c] in "APBCE"
                ps = psumC.tile([128, W], f32, tag="chunk")
                if act and L1_EXIT[c] != "B":
                    if c not in hgbs:
                        conv(c)
                    hgb = hgbs[c]
                for s0 in range(0, W, 512):
                    s1 = min(s0 + 512, W)
                    nc.tensor.matmul(out=ps[:, s0:s1], lhsT=WeT,
                                     rhs=peTA[0:32, cbase[c] + s0:cbase[c] + s1],
                                     start=True, stop=not act)
                    if act:
                        if L1_EXIT[c] in "BCE":
                            rhs = hbg_bf[:, cbase[c] + s0:cbase[c] + s1, 1:2]
                        else:
                            rhs = hgb[:, s0:s1]
                        nc.tensor.matmul(out=ps[:, s0:s1], lhsT=ident,
                                         rhs=rhs, start=False, stop=True)
                r = scrp.tile([128, CHI, S[c]], bf16, tag=f"r{S[c]}")
                r2 = r[:].rearrange("p a b -> p (a b)")
                if act:
                    if L1_EXIT[c] == "C":
                        nc.vector.tensor_scalar(out=r2, in0=ps[:],
                                                scalar1=0.0, scalar2=None,
                                                op0=MAX)
                    elif L1_EXIT[c] == "E":
                        h = W // 2
                        nc.scalar.activation(out=r2[:, 0:h], in_=ps[:, 0:h],
                                             func=RELU)
                        nc.vector.tensor_scalar(out=r2[:, h:W], in0=ps[:, h:W],
                                                scalar1=0.0, scalar2=None,
                                                op0=MAX)
                    else:
                        nc.scalar.activation(out=r2, in_=ps[:], func=RELU)
                else:
                    nc.vector._custom_dve(
                        relu_add, out=r2, in0=ps[:],
                        in1=hbg[:, cbase[c]:cbase[c] + W],
                        accum_out=dve_scrap[:])
                fold(r, c, S[c], msg, TREE1[c])
            return finish(1, msg, xsT_l)

        def x0(h):
            return xTA[:, 512 * h:512 * (h + 1)]

        # ---------------- wiring ----------------
        if mode == "l1":
            h2T = layer0((xTA, N))
        elif mode == "nocc":
            hbg1, hgbs1, conv1 = hb_prep(1, x0, 0)  # hoisted
            h1T = layer0((xTA, N))
            h2T = layer1(1, hbg1, hgbs1, conv1, (h1T, 0))
        elif mode == "full":
            h1T = layer0((xTA, N))
            # ------------- allgather updated node features -------------
            ps_t = psumF.tile([SH, D], bf16, tag="fin")
            nc.tensor.transpose(ps_t[:], h1T[:], ident)
            h1_own = P.tile([SH, D], f32)
            nc.scalar.copy(h1_own[:], ps_t[:])

            gin = dramp.tile([SH, D], f32)
            gout = dramp.tile([N, D], f32)
            nc.gpsimd.dma_start(out=gin[:], in_=h1_own[:])
            nc.gpsimd.collective_compute(
                "AllGather", mybir.AluOpType.bypass,
                replica_groups=[list(range(NC))],
                ins=[gin[:].opt()], outs=[gout[:].opt()])

            x1b = P.tile([128, N // 128, D], bf16)
            nc.gpsimd.dma_start(
                out=x1b[:], in_=gout[:].rearrange("(jt p) d -> p jt d", p=128))
            x1T = P.tile([D, N // 128, 128], bf16)
            nc.sync.dma_start(out=x1T[:], in_=x1b[:], transpose=True)

            def x1(h):
                return x1T[:, 4 * h:4 * (h + 1), :]

            hbg1, hgbs1, conv1 = hb_prep(1, x1, 1)
            h2T = layer1(1, hbg1, hgbs1, conv1, (h1T, 0))
        else:
            raise ValueError(mode)

        # ---------------- output (rows in sorted order) ----------------
        h2_own = P.tile([SH, D], f32)
        for hh in range(2):
            sl = slice(64 * hh, 64 * (hh + 1))
            ps_o = psumF.tile([64, D], bf16, tag="fin")
            nc.tensor.transpose(ps_o[:], h2T[:, sl], ident)
            nc.vector.tensor_scalar(out=h2_own[sl, :], in0=ps_o[:],
                                    scalar1=0.0, scalar2=None, op0=ADD)
            nc.sync.dma_start(out=out_d[sl, :], in_=h2_own[sl, :])

    nc.compile()
    return nc


def _plan(adj):
    """Degree-sort rows per core, bucket into NCH chunks, pad to mult of 4."""
    deg = adj.sum(1).astype(np.int64).reshape(NC, SH)
    perms = [np.argsort(-deg[c], kind="stable") for c in range(NC)]
    S = []
    for ch in range(NCH):
        mx = max(int(deg[c][perms[c][CHI * ch:CHI * (ch + 1)]].max())
                 for c in range(NC))
        S.append(max(4, int(-(-mx // 4) * 4)))
    return perms, tuple(S)


def _wrap_idx(L):
    """ap_gather index layout: [128, Q//16], idx[p, m] = L[m*16 + p%16]."""
    w = L.reshape(-1, 16).T.astype(np.int16)          # [16, Q//16]
    return np.tile(w, (8, 1))


def _host_inputs(inputs):
    """Build the 8 per-core input maps + plan from full inputs."""
    import ml_dtypes

    bf = ml_dtypes.bfloat16
    adj = np.asarray(inputs["adj"], np.float32)
    nodes = np.asarray(inputs["nodes"], np.float32)
    edges = np.asarray(inputs["edges"], np.float32)
    eps = float(np.asarray(inputs["eps"], np.float32).reshape(-1)[0])
    perms, S = _plan(adj)
    Q = CHI * sum(S)
    cbase = [CHI * sum(S[:c]) for c in range(NCH)]

    # global position of node j in the allgathered (per-core sorted) layout
    gpos = np.empty(N, np.int64)
    for c in range(NC):
        gpos[c * SH + perms[c]] = c * SH + np.arange(SH)

    Wne = [np.asarray(inputs["Wne0"], np.float32),
           np.asarray(inputs["Wne1"], np.float32)]
    Wb = np.concatenate(
        [np.concatenate(
            [Wne[l][:, :D].T,
             np.asarray(inputs[f"Wn{l}"], np.float32).T], axis=1)
         for l in range(2)] + [np.eye(D, dtype=np.float32)], axis=1)
    # WeA: [WeT_l ; bne_l ; ones] stacked per layer -> [34, 2D]
    WeA = np.zeros((34, 2 * D), np.float32)
    for l in range(2):
        WeA[0:32, D * l:D * (l + 1)] = Wne[l][:, D:D + E].T
        WeA[32, D * l:D * (l + 1)] = np.asarray(inputs[f"bne{l}"], np.float32)
        WeA[33, D * l:D * (l + 1)] = 1.0
    bias = np.stack(
        [np.asarray(inputs["bne0"], np.float32),
         np.asarray(inputs["bn0"], np.float32),
         np.asarray(inputs["bne1"], np.float32),
         np.asarray(inputs["bn1"], np.float32),
         np.full(D, 1.0 + eps, np.float32)], axis=1)
    com = {
        "Wb": np.ascontiguousarray(Wb.astype(bf)),
        "WeA": np.ascontiguousarray(WeA.astype(bf)),
        "bias": np.ascontiguousarray(bias),
    }

    maps = []
    for c in range(NC):
        perm = perms[c]
        rows = c * SH + perm                       # global ids, sorted order
        pea = np.zeros((Q, 34), np.float32)
        pea[:, 33] = -1e9                          # pad mask row
        L1 = np.full(Q, PAD, np.int64)
        L2 = np.full(Q, PAD, np.int64)
        xg = np.zeros((Q, D), np.float32)
        for p in range(SH):
            ch = p // CHI
            il = p % CHI
            base = cbase[ch] + il * S[ch]
            nbr = np.nonzero(adj[rows[p]])[0]
            k = len(nbr)
            assert k <= S[ch]
            pea[base:base + k, 0:E] = edges[rows[p], nbr]
            pea[base:base + k, 32] = 1.0           # bias carrier
            pea[base:base + k, 33] = 0.0           # not padded
            xg[base:base + k] = nodes[nbr]
            L1[base:base + k] = nbr
            L2[base:base + k] = gpos[nbr]
        m = dict(com)
        m["xT"] = np.ascontiguousarray(
            np.concatenate([nodes.T, nodes[rows].T], axis=1).astype(bf))
        m["peTA_sh"] = np.ascontiguousarray(pea.T.astype(bf))
        xdt = ml_dtypes.float8_e4m3 if XG_FP8 else bf
        m["xgT_sh"] = np.ascontiguousarray(xg.T.astype(xdt))
        m["idx_sh"] = np.ascontiguousarray(
            np.concatenate([_wrap_idx(L1), _wrap_idx(L2)], axis=1))
        maps.append(m)
    return maps, perms, S


def _get_runner(S):
    """Build (once per S) a cached jit(shard_map) callable."""
    key = ("runner", S)
    if key in _cache:
        return _cache[key]
    import jax
    from jax.sharding import Mesh, PartitionSpec, NamedSharding
    from jax.experimental.shard_map import shard_map
    import concourse.mybir as mybir
    from concourse import bass2jax
    from concourse.bass2jax import _bass_exec_p, partition_id_tensor

    nckey = ("nc", S)
    if nckey not in _cache:
        _cache[nckey] = _build_nc("full", S)
    nc = _cache[nckey]
    bass2jax.install_neuronx_cc_hook()

    in_names, out_names, out_avals, zero_outs = [], [], [], []
    partition_name = nc.partition_id_tensor.name if nc.partition_id_tensor else None
    for alloc in nc.m.functions[0].allocations:
        if not isinstance(alloc, mybir.MemoryLocationSet):
            continue
        name = alloc.memorylocations[0].name
        if alloc.kind == "ExternalInput":
            if name != partition_name:
                in_names.append(name)
        elif alloc.kind == "ExternalOutput":
            shape = list(alloc.tensor_shape)
            dtype = np.dtype(mybir.dt.np(alloc.dtype))
            out_avals.append(jax.core.ShapedArray(shape, dtype))
            out_names.append(name)
            zero_outs.append(np.zeros(shape, dtype))

    n_params = len(in_names)
    all_in_names = list(in_names) + list(out_names)
    if partition_name is not None:
        all_in_names.append(partition_name)

    def _body(*args):
        operands = list(args)
        if partition_name is not None:
            operands.append(partition_id_tensor())
        outs = _bass_exec_p.bind(
            *operands,
            out_avals=tuple(out_avals),
            in_names=tuple(all_in_names),
            out_names=tuple(out_names),
            lowering_input_output_aliases=(),
            sim_require_finite=True,
            sim_require_nnan=True,
            nc=nc,
        )
        return tuple(outs)

    devices = jax.devices()[:NC]
    mesh = Mesh(np.asarray(devices), ("core",))
    n_outs = len(out_names)
    fn = jax.jit(
        shard_map(_body, mesh=mesh,
                  in_specs=(PartitionSpec("core"),) * (n_params + n_outs),
                  out_specs=(PartitionSpec("core"),) * n_outs,
                  check_rep=False),
        keep_unused=True)
    sh = NamedSharding(mesh, PartitionSpec("core"))
    dev_zeros = [
        jax.device_put(np.zeros((NC * z.shape[0], *z.shape[1:]), z.dtype), sh)
        for z in zero_outs
    ]

    def run(maps):
        dev_in = []
        for nm in in_names:
            arrs = [
                jax.device_put(np.asarray(maps[c][nm]), devices[c])
                for c in range(NC)
            ]
            shp = arrs[0].shape
            glob = jax.make_array_from_single_device_arrays(
                (NC * shp[0], *shp[1:]), sh, arrs)
            dev_in.append(glob)
        outs = fn(*dev_in, *dev_zeros)
        oi = out_names.index("out")
        return np.asarray(outs[oi]).reshape(NC, SH, D)

    _cache[key] = run
    return run


def kernel(**inputs):
    maps, perms, S = _host_inputs(inputs)
    run = _get_runner(S)
    raw = run(maps)                                # [NC, SH, D], sorted rows
    out = np.empty((N, D), np.float32)
    for c in range(NC):
        out[c * SH + perms[c]] = raw[c]
    return np.ascontiguousarray(out.astype(np.float32))


if __name__ == "__main__":
    _build_nc("nocc")
    print("build+compile OK")



# revision 28
# speedup vs baseline: 1.0852x; 1.0852x over previous
"""DGINConv (2-layer GIN with edge features) Trainium2 kernel — fp8 DoubleRow.

Math (per layer, reference):
    ne[i,j,:] = relu(Wnode@x[j] + We@edges[i,j,:] + bne)
    msg[i,:]  = sum_{j: adj[i,j]=1} ne[i,j,:]
    h[i,:]    = relu(Wn @ ((1+eps)*x[i] + msg[i]) + bn)

adj density ~3%: each own row's ~31 neighbors are packed into padded slots
(host-side): own rows degree-sorted, grouped into 8 chunks of 16 rows,
chunk c padded to S_c slots/row (mult of 4). Q = 16*sum(S_c) per core.

Key structure (vs the previous bf16 version):
- Per layer0 chunk piece, node term + edge term + bias + pad-mask are ONE
  fp8 DoubleRow matmul: rhs peT [81, 2, W] (logical row r=2k+t: 32 edge
  rows ++ bias carrier ++ mask(-16) ++ 128 host-gathered x rows fp8),
  lhsT Wcomb [81, 2, 128] fp8. DoubleRow = 0.5 PE cycles/slot.
- Layer1: edge+bias+mask rows are the SAME peT partitions 0:17 (row
  interleave chosen so logical rows 0..33 = partitions 0..16 x 2) with
  layer-1 weights; the node term gathers allgathered features per slot
  (hybrid: dma_gather straight from DRAM bf16 rows -> [128, n] SBUF,
  rest ap_gather from an f32 SBUF copy) + one bf16 K=128 matmul into the
  same PSUM. No hbT, no identity-inject.
- Exits per chunk (knob): 'A' = ACT relu + DVE pairwise tree fold;
  'V' = custom DVE op relu(a)+relu(b) fusing exit + first tree level.
- finish l0 stays [D, rows]; finish l1 is emitted flipped (lhsT=z,
  rhs=WnT1, bias preloaded into PSUM via K=1 matmul) so output lands
  [rows, D] and DMAs straight out. No transposes anywhere in nocc.

Distribution: destination rows sharded 8 ways; nodes/weights replicated;
updated node features exchanged between layers via AllGather (full mode).
Final output rows un-permuted on the host.
"""

import sys

if "/opt/trn_rl_repo" not in sys.path:
    sys.path.insert(0, "/opt/trn_rl_repo")

import numpy as np

N, D, E, NC = 1024, 128, 32, 8
SH = N // NC          # 128 rows per core
NCH = 8               # chunks of sorted own-rows
CHI = SH // NCH       # 16 rows per chunk
KP = 81               # DoubleRow partitions: 2*81 = 162 >= 32+2+128
MASKW = 16.0          # mask weight; mask rhs = -16 -> psum -256 on pads

S_DEFAULT = (52, 40, 36, 36, 32, 32, 28, 28)

# ---------------- tuning knobs (overridable via set_cfg) ----------------
DG = ()                       # chunks gathered via dma_gather (DRAM bf16)
AP_GROUPS = ((0, 1), (2, 3), (4, 5), (6, 7))  # ap_gather call groups
PREP_POS = 0                  # index in pool sequence where dma_gather goes
L0_EXIT = "HAHAHAHA"          # A=ACT relu | D=DVE relu | H=ACT half + fused
L1_EXIT = "AAAAHHHH"          # DVE relu+add | P=Pool relu (layer1 only)
TREE1 = "PPPPPPPP"            # layer1 t2 engine: D=DVE, P=Pool
L1_ORDER = (0, 1, 2, 3, 4, 5, 6, 7)   # layer1 chunk processing order
FIN0 = "A"                    # layer0 finish relu engine: A=ACT, D=DVE
FIN1 = "D"                    # layer1 finish relu engine
FIN1_ORDER = (0, 1)           # finish1 half emission order
OUT = "S"                     # output: T=SWDGE prep+trigger scatter, S=dma
PIECE = 256                   # DoubleRow piece width (<=256)
# DMA issue plan: queue S=nc.sync, C=nc.scalar, V=nc.vector; names below
DMA_PLAN = (
    ("S", "Wcomb"), ("S", "peT0"), ("S", "idx"), ("S", "xe32"),
    ("S", "peT1"), ("S", "peT2"), ("S", "peT3"),
    ("S", "WnB"), ("S", "bias"), ("S", "xown"),
)

_KNOBS = ("DG", "AP_GROUPS", "PREP_POS", "L0_EXIT", "L1_EXIT", "TREE1",
          "L1_ORDER", "FIN0", "FIN1", "FIN1_ORDER", "OUT", "PIECE", "DMA_PLAN")


def set_cfg(**kw):
    g = globals()
    for k, v in kw.items():
        assert k in _KNOBS, k
        g[k] = v

_cache = {}
_CUSTOM = {}


def _ensure_relu2add():
    """Register RELUADD1_GIN: out = relu(in0) + in1; accum = sum.

    in0 may be PSUM (the un-relu'd second half of a chunk); in1 is the
    already-relu'd first half in SBUF — only one PSUM operand, which is
    all the ISA allows.
    """
    if "op" in _CUSTOM:
        return _CUSTOM["op"]
    import concourse.dve_ops as dve_ops
    from concourse.dve_spec import Spec, Src0, Src1, relu, lower, _has_src1
    from concourse.dve_spec import Zero
    from concourse.dve_uop import DveOpSpec
    from operator import add

    name = "RELUADD1_GIN"

    def _ref(in0, in1, c0, c1, c2):
        b = (dve_ops._dve_relu(in0.astype(np.float32))
             + in1.astype(np.float32).reshape(in0.shape))
        return b, b.reshape(b.shape[0], -1).sum(axis=-1, keepdims=True)

    spec = Spec(body=relu(Src0) + Src1, accum=add, accum_init=Zero,
                reference=_ref)
    row = dve_ops._CUSTOM_DVE_ROW_BASE + len(dve_ops.OPS)
    assert row < 0x20
    shas = {}
    for ver in ("v3", "v4"):
        try:
            s = DveOpSpec(name=name, opcode=row, uops=lower(spec, ver=ver),
                          rd1_en=_has_src1(spec))
            shas[ver] = s.sha(ver)
        except Exception:
            pass
    op = dve_ops.DveOp(name, spec, subdim=False, uops_sha=shas)
    dve_ops.OPS.append(op)
    dve_ops.CUSTOM_DVE_SPECS[name] = spec
    dve_ops._SUB_OPCODE_FOR_NAME[name] = row
    _CUSTOM["op"] = op
    return op


def _spaces(S):
    """Derive slot-space geometry from chunk sizes."""
    S = tuple(S)
    Q = CHI * sum(S)
    cbase = [CHI * sum(S[:c]) for c in range(NCH)]
    apch = [c for g in AP_GROUPS for c in g]
    Qa = CHI * sum(S[c] for c in apch)
    apo = {}
    off = 0
    for c in apch:
        apo[c] = off
        off += CHI * S[c]
    Qd = CHI * sum(S[c] for c in DG)
    Qdp = -(-Qd // 128) * 128
    dgo = {}
    off = 0
    for c in DG:
        dgo[c] = off
        off += CHI * S[c]
    return Q, cbase, Qa, apo, Qd, Qdp, dgo


def _build_nc(mode="full", S=S_DEFAULT):
    from contextlib import ExitStack

    import concourse.mybir as mybir
    import concourse.tile as tile
    from concourse import bacc

    f32 = mybir.dt.float32
    bf16 = mybir.dt.bfloat16
    fp8 = mybir.dt.float8e4
    i16 = mybir.dt.int16
    RELU = mybir.ActivationFunctionType.Relu
    ADD = mybir.AluOpType.add
    MAX = mybir.AluOpType.max
    MULT = mybir.AluOpType.mult
    DR = mybir.MatmulPerfMode.DoubleRow

    relu2add = _ensure_relu2add()
    S = tuple(S)
    Q, cbase, Qa, apo, Qd, Qdp, dgo = _spaces(S)
    IW = Qa // 16 + Qdp // 16 + SH // 16

    nc = bacc.Bacc("TRN2", target_bir_lowering=False, debug=False,
                   enable_asserts=False, num_devices=NC)

    def din(name, shape, dt=None):
        return nc.dram_tensor(name, shape, dt or f32, kind="ExternalInput").ap()

    peT_d = din("peT_sh", [KP, 2, Q], fp8)
    Wcomb_d = din("Wcomb", [KP, 2, 2 * D], fp8)
    WnB_d = din("WnB", [D, 4 * D + 64], bf16)  # Wn1T|WnT0|WnT1|row0: ones,bn1
    bias_d = din("bias", [D, 4])               # bn0 | 1+eps
    xown_d = din("xown_sh", [D, SH])
    idx_d = din("idx_sh", [128, IW], i16)
    if mode != "full":
        xe32_d = din("xe32_sh", [D, N])        # f32 allgathered-x stand-in
        xgou_d = din("xgou_sh", [N, D], bf16)  # node-major bf16 stand-in
    out_d = nc.dram_tensor("out", [SH, D], f32, kind="ExternalOutput").ap()

    with tile.TileContext(nc) as tc, ExitStack() as ctx:
        P = ctx.enter_context(tc.tile_pool(name="persist", bufs=1))
        dramp = ctx.enter_context(tc.tile_pool(name="dram", bufs=1, space="DRAM"))
        psumC = ctx.enter_context(tc.tile_pool(name="psumC", bufs=3, space="PSUM"))
        psumF = ctx.enter_context(tc.tile_pool(name="psumF", bufs=1, space="PSUM"))
        scrp = ctx.enter_context(tc.tile_pool(name="scr", bufs=3))

        # ---------------- input DMAs ----------------
        peT = P.tile([KP, 2, Q], fp8)
        Wcomb = P.tile([KP, 2, 2 * D], fp8)
        WnB = P.tile([D, 4 * D + 64], bf16)
        bias = P.tile([D, 4], f32)
        xown = P.tile([D, SH], f32)
        idx = P.tile([128, IW], i16)
        xe32 = P.tile([D, N], f32)

        qmap = {"S": nc.sync, "C": nc.scalar, "V": nc.vector}
        pbounds = [0, cbase[2], cbase[4], cbase[6], Q]

        def issue(q, name):
            eng = qmap[q]
            if name.startswith("peT"):
                i = int(name[3])
                lo, hi = pbounds[i], pbounds[i + 1]
                eng.dma_start(out=peT[:, :, lo:hi], in_=peT_d[:, :, lo:hi])
            elif name == "xe32":
                if mode != "full":
                    eng.dma_start(out=xe32[:], in_=xe32_d[:])
            elif name == "Wcomb":
                eng.dma_start(out=Wcomb[:], in_=Wcomb_d[:])
            elif name == "WnB":
                eng.dma_start(out=WnB[:], in_=WnB_d[:])
            elif name == "bias":
                eng.dma_start(out=bias[:], in_=bias_d[:])
            elif name == "xown":
                eng.dma_start(out=xown[:], in_=xown_d[:])
            elif name == "idx":
                eng.dma_start(out=idx[:], in_=idx_d[:])

        for q, name in DMA_PLAN:
            issue(q, name)

        dve_scrap = P.tile([128, 1], f32)
        Wn1T = WnB[:, 0:D]
        WnT0 = WnB[:, D:2 * D]
        WnT1 = WnB[:, 2 * D:3 * D]
        ones64 = WnB[0:1, 3 * D:3 * D + 64]
        bn1r = WnB[0:1, 3 * D + 64:4 * D + 64]
        bn0 = bias[:, 0:1]
        opse = bias[:, 1:2]

        # ---------------- exit + fold ----------------
        def exit_fold(c, ps, msg, ex, tr="D"):
            """PSUM [128, CHI*S[c]] -> relu -> segment sum -> msg cols."""
            Sc = S[c]
            W = CHI * Sc
            h = Sc // 2
            ps_r = ps[:].rearrange("p (a b) -> p a b", a=CHI)
            if ex == "H":
                # ACT relus the first half-slots; DVE fuses relu of the
                # PSUM second half with the add (one PSUM operand only).
                r1 = scrp.tile([128, CHI, h], bf16, tag=f"r1{Sc}")
                nc.scalar.activation(out=r1[:], in_=ps_r[:, :, 0:h],
                                     func=RELU)
                t1 = scrp.tile([128, CHI, h], bf16, tag=f"t1{Sc}")
                nc.vector._custom_dve(
                    relu2add, out=t1[:], in0=ps_r[:, :, h:Sc],
                    in1=r1[:], accum_out=dve_scrap[:])
            else:
                r = scrp.tile([128, CHI, Sc], bf16, tag=f"r{Sc}")
                if ex == "A":
                    nc.scalar.activation(
                        out=r[:].rearrange("p a b -> p (a b)"),
                        in_=ps[:, 0:W], func=RELU)
                elif ex == "D":
                    nc.vector.tensor_scalar(
                        out=r[:].rearrange("p a b -> p (a b)"),
                        in0=ps[:, 0:W], scalar1=0.0, scalar2=None, op0=MAX)
                else:
                    nc.gpsimd.tensor_scalar(
                        out=r[:].rearrange("p a b -> p (a b)"),
                        in0=ps[:, 0:W], scalar1=0.0, scalar2=None, op0=MAX)
                t1 = scrp.tile([128, CHI, h], bf16, tag=f"t1{Sc}")
                nc.vector.tensor_tensor(out=t1[:], in0=r[:, :, 0:h],
                                        in1=r[:, :, h:Sc], op=ADD)
            h2 = h // 2
            t2 = scrp.tile([128, CHI, h2], bf16, tag=f"t2{Sc}")
            teng = nc.gpsimd if tr == "P" else nc.vector
            teng.tensor_tensor(out=t2[:], in0=t1[:, :, 0:h2],
                               in1=t1[:, :, h2:h], op=ADD)
            reng = nc.gpsimd if tr == "Q" else nc.vector
            reng.tensor_reduce(
                out=msg[:, CHI * c:CHI * (c + 1)], in_=t2[:],
                axis=mybir.AxisListType.X, op=ADD)

        # ---------------- layer 0 ----------------
        def layer0():
            W0 = Wcomb[:, :, 0:D]
            msg = P.tile([D, SH], f32, tag="msg0")
            for c in range(NCH):
                W = CHI * S[c]
                ps = psumC.tile([128, W], f32, tag="chunk")
                for s0 in range(0, W, PIECE):
                    s1 = min(s0 + PIECE, W)
                    nc.tensor.matmul(
                        out=ps[:, s0:s1], lhsT=W0,
                        rhs=peT[:, :, cbase[c] + s0:cbase[c] + s1],
                        start=True, stop=True, perf_mode=DR)
                exit_fold(c, ps, msg, L0_EXIT[c])
            return msg

        def finish0(msg):
            h1T = P.tile([D, SH], f32, tag="h1T")
            for hh in range(2):
                sl = slice(64 * hh, 64 * (hh + 1))
                z = scrp.tile([D, 64], bf16, tag=f"z0{hh}")
                nc.vector.scalar_tensor_tensor(
                    out=z[:], in0=xown[:, sl], scalar=opse,
                    in1=msg[:, sl], op0=MULT, op1=ADD)
                ps = psumF.tile([D, 64], f32, tag="fin0")
                nc.tensor.matmul(out=ps[:], lhsT=WnT0, rhs=z[:],
                                 start=True, stop=True)
                if FIN0 == "A":
                    nc.scalar.activation(out=h1T[:, sl], in_=ps[:],
                                         func=RELU, bias=bn0)
                else:
                    nc.vector.tensor_scalar(out=h1T[:, sl], in0=ps[:],
                                            scalar1=bn0, scalar2=0.0,
                                            op0=ADD, op1=MAX)
            return h1T

        # ---------------- gathers (layer 1 node features) ----------------
        def gathers(xgou_src):
            xg1f = P.tile([128, max(Qa, 16)], f32)
            xg1b = P.tile([128, 1, max(Qdp, 128)], bf16)
            plan = []
            for gi, grp in enumerate(AP_GROUPS):
                plan.append(("ap", gi, grp))
            if DG:
                plan.insert(PREP_POS, ("dg",))
            for item in plan:
                if item[0] == "dg":
                    nc.gpsimd.dma_gather(
                        out_ap=xg1b[:, :, 0:Qdp], in_ap=xgou_src,
                        idxs_ap=idx[:, Qa // 16:Qa // 16 + Qdp // 16],
                        num_idxs=Qdp, num_idxs_reg=Qdp, elem_size=D,
                        transpose=True)
                else:
                    _, gi, grp = item
                    lo = apo[grp[0]]
                    hi = apo[grp[-1]] + CHI * S[grp[-1]]
                    nc.gpsimd.ap_gather(
                        out_ap=xg1f[:, lo:hi], in_ap=xe32[:, 0:N],
                        idxs_ap=idx[:, lo // 16:hi // 16],
                        channels=128, num_elems=N, d=1, num_idxs=hi - lo)
            return xg1f, xg1b

        # ---------------- layer 1 ----------------
        def layer1(xg1f, xg1b):
            W1e = Wcomb[0:17, :, D:2 * D]
            xgb = xg1f[:].bitcast(bf16).rearrange(
                "p (q two) -> p q two", two=2)
            msg = P.tile([D, SH], f32, tag="msg1")
            for c in L1_ORDER:
                W = CHI * S[c]
                ps = psumC.tile([128, W], f32, tag="chunk")
                for s0 in range(0, W, PIECE):
                    s1 = min(s0 + PIECE, W)
                    nc.tensor.matmul(
                        out=ps[:, s0:s1], lhsT=W1e,
                        rhs=peT[0:17, :, cbase[c] + s0:cbase[c] + s1],
                        start=True, stop=False, perf_mode=DR)
                    if c in dgo:
                        rhs = xg1b[:, 0, dgo[c] + s0:dgo[c] + s1]
                    else:
                        rhs = xgb[:, apo[c] + s0:apo[c] + s1, 1:2]
                    nc.tensor.matmul(out=ps[:, s0:s1], lhsT=Wn1T, rhs=rhs,
                                     start=False, stop=True)
                exit_fold(c, ps, msg, L1_EXIT[c], TREE1[c])
            return msg

        def finish1(msg, h1T, h2own):
            for hh in FIN1_ORDER:
                sl = slice(64 * hh, 64 * (hh + 1))
                z = scrp.tile([D, 64], bf16, tag=f"z1{hh}")
                nc.vector.scalar_tensor_tensor(
                    out=z[:], in0=h1T[:, sl], scalar=opse,
                    in1=msg[:, sl], op0=MULT, op1=ADD)
                ps = psumF.tile([64, D], f32, tag="fin1")
                nc.tensor.matmul(out=ps[:], lhsT=ones64, rhs=bn1r,
                                 start=True, stop=False)
                nc.tensor.matmul(out=ps[:], lhsT=z[:], rhs=WnT1,
                                 start=False, stop=True)
                if FIN1 == "A":
                    nc.scalar.activation(out=h2own[sl, :], in_=ps[:],
                                         func=RELU)
                else:
                    nc.vector.tensor_scalar(out=h2own[sl, :], in0=ps[:],
                                            scalar1=0.0, scalar2=None,
                                            op0=MAX)
                if OUT != "T":
                    nc.sync.dma_start(out=out_d[sl, :], in_=h2own[sl, :])
            if OUT == "T":
                nc.gpsimd.trigger_dma(count=1, queue_num=0)

        # ---------------- wiring ----------------
        h2own = P.tile([SH, D], f32)

        def out_prep():
            if OUT == "T":
                out_sem = nc.alloc_semaphore("out_dma")
                nc.gpsimd.dma_scatter_add(
                    out_d[:],
                    h2own[:].rearrange("p (g d) -> p g d", g=1),
                    idx[:, IW - SH // 16:IW],
                    SH, SH, D,
                    prepare_only=True, sem=out_sem, queue_num=0)

        if mode == "full":
            msg0 = layer0()
            h1T = finish0(msg0)
            gout = None
            if DG:
                # h1 rows (bf16, [SH, D]) for the dma_gather source
                h1Tb = P.tile([D, SH], bf16)
                nc.scalar.activation(out=h1Tb[:], in_=h1T[:],
                                     func=mybir.ActivationFunctionType.Identity)
                h1r = P.tile([SH, D], bf16)
                nc.sync.dma_start(out=h1r[:], in_=h1Tb[:], transpose=True)
                gin = dramp.tile([SH, D], bf16)
                gout = dramp.tile([N, D], bf16)
                nc.gpsimd.dma_start(out=gin[:], in_=h1r[:])
                nc.gpsimd.collective_compute(
                    "AllGather", mybir.AluOpType.bypass,
                    replica_groups=[list(range(NC))],
                    ins=[gin[:].opt()], outs=[gout[:].opt()])
            if AP_GROUPS:
                gin2 = dramp.tile([D, SH], f32)
                gout2 = dramp.tile([NC * D, SH], f32)
                nc.gpsimd.dma_start(out=gin2[:], in_=h1T[:])
                nc.gpsimd.collective_compute(
                    "AllGather", mybir.AluOpType.bypass,
                    replica_groups=[list(range(NC))],
                    ins=[gin2[:].opt()], outs=[gout2[:].opt()])
                nc.sync.dma_start(
                    out=xe32[:].rearrange("p (c i) -> p c i", c=NC),
                    in_=gout2[:].rearrange("(c d) i -> d c i", d=D))
            xg1f, xg1b = gathers(gout[:] if gout is not None else None)
            out_prep()
            msg1 = layer1(xg1f, xg1b)
            finish1(msg1, h1T, h2own)
        else:
            # timed variant: no collective; gather source is a host tensor
            xg1f, xg1b = gathers(xgou_d[:] if DG else None)
            out_prep()
            msg0 = layer0()
            h1T = finish0(msg0)
            msg1 = layer1(xg1f, xg1b)
            finish1(msg1, h1T, h2own)

    nc.compile()
    return nc


def _plan(adj):
    """Degree-sort rows per core, bucket into NCH chunks, pad to mult of 4."""
    deg = adj.sum(1).astype(np.int64).reshape(NC, SH)
    perms = [np.argsort(-deg[c], kind="stable") for c in range(NC)]
    S = []
    for ch in range(NCH):
        mx = max(int(deg[c][perms[c][CHI * ch:CHI * (ch + 1)]].max())
                 for c in range(NC))
        S.append(max(4, int(-(-mx // 4) * 4)))
    return perms, tuple(S)


def _wrap_idx(L):
    """gather index layout: [128, n//16], idx[p, m] = L[m*16 + p%16]."""
    w = np.asarray(L).reshape(-1, 16).T.astype(np.int16)
    return np.tile(w, (8, 1))


def _host_inputs(inputs):
    """Build the 8 per-core input maps + plan from full inputs."""
    import ml_dtypes

    bf = ml_dtypes.bfloat16
    f8 = ml_dtypes.float8_e4m3
    adj = np.asarray(inputs["adj"], np.float32)
    nodes = np.asarray(inputs["nodes"], np.float32)
    edges = np.asarray(inputs["edges"], np.float32)
    eps = float(np.asarray(inputs["eps"], np.float32).reshape(-1)[0])
    perms, S = _plan(adj)
    Q, cbase, Qa, apo, Qd, Qdp, dgo = _spaces(S)

    # global position of node j in the allgathered (per-core sorted) layout
    gpos = np.empty(N, np.int64)
    for c in range(NC):
        gpos[c * SH + perms[c]] = c * SH + np.arange(SH)
    sorted_nodes = np.empty((N, D), np.float32)
    for c in range(NC):
        sorted_nodes[c * SH:(c + 1) * SH] = nodes[c * SH + perms[c]]

    Wne = [np.asarray(inputs["Wne0"], np.float32),
           np.asarray(inputs["Wne1"], np.float32)]
    bne = [np.asarray(inputs["bne0"], np.float32),
           np.asarray(inputs["bne1"], np.float32)]
    # Wcomb: logical contraction rows (interleaved r=2k+t):
    #   0..31 edge rows, 32 bias carrier, 33 mask, 34..161 node rows (l0)
    Wc = np.zeros((2 * KP, 2 * D), np.float32)
    for l in range(2):
        Wc[0:E, l * D:(l + 1) * D] = Wne[l][:, D:D + E].T
        Wc[E, l * D:(l + 1) * D] = bne[l]
        Wc[E + 1, l * D:(l + 1) * D] = MASKW
    Wc[E + 2:E + 2 + D, 0:D] = Wne[0][:, :D].T
    Wcomb = np.clip(Wc, -440, 440).reshape(KP, 2, 2 * D)

    WnB = np.zeros((D, 4 * D + 64), np.float32)
    WnB[:, 0:D] = Wne[1][:, :D].T
    WnB[:, D:2 * D] = np.asarray(inputs["Wn0"], np.float32).T
    WnB[:, 2 * D:3 * D] = np.asarray(inputs["Wn1"], np.float32).T
    WnB[0, 3 * D:3 * D + 64] = 1.0
    WnB[0, 3 * D + 64:4 * D + 64] = np.asarray(inputs["bn1"], np.float32)

    bias = np.zeros((D, 4), np.float32)
    bias[:, 0] = np.asarray(inputs["bn0"], np.float32)
    bias[:, 1] = 1.0 + eps

    com = {
        "Wcomb": np.ascontiguousarray(Wcomb.astype(f8)),
        "WnB": np.ascontiguousarray(WnB.astype(bf)),
        "bias": np.ascontiguousarray(bias),
    }

    apch = [c for g in AP_GROUPS for c in g]
    maps = []
    for c in range(NC):
        perm = perms[c]
        rows = c * SH + perm
        pe = np.zeros((Q, 2 * KP), np.float32)
        pe[:, E + 1] = -MASKW                     # mask row: pads -16
        La = np.zeros(Qa, np.int64)
        Ld = np.zeros(Qdp, np.int64)
        for p in range(SH):
            ch = p // CHI
            il = p % CHI
            base = cbase[ch] + il * S[ch]
            nbr = np.nonzero(adj[rows[p]])[0]
            k = len(nbr)
            assert k <= S[ch]
            pe[base:base + k, 0:E] = edges[rows[p], nbr]
            pe[base:base + k, E] = 1.0            # bias carrier
            pe[base:base + k, E + 1] = 0.0        # not padded
            pe[base:base + k, E + 2:E + 2 + D] = nodes[nbr]
            sbase = (dgo[ch] if ch in dgo else apo[ch]) + il * S[ch]
            tgt = Ld if ch in dgo else La
            tgt[sbase:sbase + k] = gpos[nbr]
        m = dict(com)
        m["peT_sh"] = np.ascontiguousarray(
            np.clip(pe.T, -440, 440).reshape(KP, 2, Q).astype(f8))
        m["xown_sh"] = np.ascontiguousarray(nodes[rows].T)
        m["idx_sh"] = np.ascontiguousarray(
            np.concatenate([_wrap_idx(La), _wrap_idx(Ld),
                            _wrap_idx(np.arange(SH))], axis=1))
        m["xe32_sh"] = np.ascontiguousarray(sorted_nodes.T)
        m["xgou_sh"] = np.ascontiguousarray(sorted_nodes.astype(bf))
        maps.append(m)
    return maps, perms, S


def _get_runner(S):
    """Build (once per S) a cached jit(shard_map) callable."""
    key = ("runner", S)
    if key in _cache:
        return _cache[key]
    import jax
    from jax.sharding import Mesh, PartitionSpec, NamedSharding
    from jax.experimental.shard_map import shard_map
    import concourse.mybir as mybir
    from concourse import bass2jax
    from concourse.bass2jax import _bass_exec_p, partition_id_tensor

    nckey = ("nc", S)
    if nckey not in _cache:
        _cache[nckey] = _build_nc("full", S)
    nc = _cache[nckey]
    bass2jax.install_neuronx_cc_hook()

    in_names, out_names, out_avals, zero_outs = [], [], [], []
    partition_name = nc.partition_id_tensor.name if nc.partition_id_tensor else None
    for alloc in nc.m.functions[0].allocations:
        if not isinstance(alloc, mybir.MemoryLocationSet):
            continue
        name = alloc.memorylocations[0].name
        if alloc.kind == "ExternalInput":
            if name != partition_name:
                in_names.append(name)
        elif alloc.kind == "ExternalOutput":
            shape = list(alloc.tensor_shape)
            dtype = np.dtype(mybir.dt.np(alloc.dtype))
            out_avals.append(jax.core.ShapedArray(shape, dtype))
            out_names.append(name)
            zero_outs.append(np.zeros(shape, dtype))

    n_params = len(in_names)
    all_in_names = list(in_names) + list(out_names)
    if partition_name is not None:
        all_in_names.append(partition_name)

    def _body(*args):
        operands = list(args)
        if partition_name is not None:
            operands.append(partition_id_tensor())
        outs = _bass_exec_p.bind(
            *operands,
            out_avals=tuple(out_avals),
            in_names=tuple(all_in_names),
            out_names=tuple(out_names),
            lowering_input_output_aliases=(),
            sim_require_finite=True,
            sim_require_nnan=True,
            nc=nc,
        )
        return tuple(outs)

    devices = jax.devices()[:NC]
    mesh = Mesh(np.asarray(devices), ("core",))
    n_outs = len(out_names)
    fn = jax.jit(
        shard_map(_body, mesh=mesh,
                  in_specs=(PartitionSpec("core"),) * (n_params + n_outs),
                  out_specs=(PartitionSpec("core"),) * n_outs,
                  check_rep=False),
        keep_unused=True)
    sh = NamedSharding(mesh, PartitionSpec("core"))
    dev_zeros = [
        jax.device_put(np.zeros((NC * z.shape[0], *z.shape[1:]), z.dtype), sh)
        for z in zero_outs
    ]

    def run(maps):
        dev_in = []
        for nm in in_names:
            arrs = [
                jax.device_put(np.asarray(maps[c][nm]), devices[c])
                for c in range(NC)
            ]
            shp = arrs[0].shape
            glob = jax.make_array_from_single_device_arrays(
                (NC * shp[0], *shp[1:]), sh, arrs)
            dev_in.append(glob)
        outs = fn(*dev_in, *dev_zeros)
        oi = out_names.index("out")
        return np.asarray(outs[oi]).reshape(NC, SH, D)

    _cache[key] = run
    return run


def kernel(**inputs):
    maps, perms, S = _host_inputs(inputs)
    run = _get_runner(S)
    raw = run(maps)                                # [NC, SH, D], sorted rows
    out = np.empty((N, D), np.float32)
    for c in range(NC):
        out[c * SH + perms[c]] = raw[c]
    return np.ascontiguousarray(out.astype(np.float32))


if __name__ == "__main__":
    _build_nc("nocc")
    print("build+compile OK")


# revision 33
# speedup vs baseline: 1.1981x; 1.1040x over previous
"""DGINConv (2-layer GIN with edge features) Trainium2 kernel — fp8 DoubleRow.

Math (per layer, reference):
    ne[i,j,:] = relu(Wnode@x[j] + We@edges[i,j,:] + bne)
    msg[i,:]  = sum_{j: adj[i,j]=1} ne[i,j,:]
    h[i,:]    = relu(Wn @ ((1+eps)*x[i] + msg[i]) + bn)

adj density ~3%: each own row's ~31 neighbors are packed into padded slots
(host-side): own rows degree-sorted, grouped into 8 chunks of 16 rows,
chunk c padded to S_c slots/row (mult of 4). Q = 16*sum(S_c) per core.

Key structure (vs the previous bf16 version):
- Per layer0 chunk piece, node term + edge term + bias + pad-mask are ONE
  fp8 DoubleRow matmul: rhs peT [81, 2, W] (logical row r=2k+t: 32 edge
  rows ++ bias carrier ++ mask(-16) ++ 128 host-gathered x rows fp8),
  lhsT Wcomb [81, 2, 128] fp8. DoubleRow = 0.5 PE cycles/slot.
- Layer1: edge+bias+mask rows are the SAME peT partitions 0:17 (row
  interleave chosen so logical rows 0..33 = partitions 0..16 x 2) with
  layer-1 weights; the node term gathers allgathered features per slot
  (hybrid: dma_gather straight from DRAM bf16 rows -> [128, n] SBUF,
  rest ap_gather from an f32 SBUF copy) + one bf16 K=128 matmul into the
  same PSUM. No hbT, no identity-inject.
- Exits per chunk (knob): 'A' = ACT relu + DVE pairwise tree fold;
  'V' = custom DVE op relu(a)+relu(b) fusing exit + first tree level.
- finish l0 stays [D, rows]; finish l1 is emitted flipped (lhsT=z,
  rhs=WnT1, bias preloaded into PSUM via K=1 matmul) so output lands
  [rows, D] and DMAs straight out. No transposes anywhere in nocc.

Distribution: destination rows sharded 8 ways; nodes/weights replicated;
updated node features exchanged between layers via AllGather (full mode).
Final output rows un-permuted on the host.
"""

import sys

if "/opt/trn_rl_repo" not in sys.path:
    sys.path.insert(0, "/opt/trn_rl_repo")

import numpy as np

N, D, E, NC = 1024, 128, 32, 8
SH = N // NC          # 128 rows per core
NCH = 8               # chunks of sorted own-rows
CHI = SH // NCH       # 16 rows per chunk
KP = 81               # DoubleRow partitions: 2*81 = 162 >= 32+2+128
MASKW = 16.0          # mask weight; mask rhs = -16 -> psum -256 on pads

S_DEFAULT = (52, 40, 36, 36, 32, 32, 28, 28)

# ---------------- tuning knobs (overridable via set_cfg) ----------------
DG = ()                       # dma_gather unsupported on this backend
AP_GROUPS = ((0, 1), (2, 3), (4, 5), (6, 7))  # ap_gather call groups
PREP_POS = 0                  # index in pool sequence where dma_gather goes
L0_EXIT = "HAHAHAHA"          # A=ACT relu | D=DVE relu | H=ACT half + fused DVE
L1_EXIT = "HAHAHAHA"          # P=Pool relu (layer1 only)
TREE1 = "DDPPPPPP"            # layer1 t2 engine: D=DVE, P=Pool
L1_ORDER = (0, 1, 2, 3, 4, 5, 6, 7)   # layer1 chunk processing order
FIN0 = "A"                    # layer0 finish relu engine: A=ACT, D=DVE
FIN1 = "D"                    # layer1 finish relu engine
FIN1_ORDER = (0, 1)           # finish1 half emission order
OUT = "S"                     # output: T=SWDGE prep+trigger scatter, S=dma
PIECE = 256                   # DoubleRow piece width (<=256)
# DMA issue plan: queue S=nc.sync, C=nc.scalar, V=nc.vector; names below
DMA_PLAN = (
    ("S", "peT0"), ("S", "idx"), ("S", "peT1"), ("S", "xe32"),
    ("S", "peT2"), ("S", "peT3"), ("S", "WnB"), ("S", "xb"),
)

_KNOBS = ("DG", "AP_GROUPS", "PREP_POS", "L0_EXIT", "L1_EXIT", "TREE1",
          "L1_ORDER", "FIN0", "FIN1", "FIN1_ORDER", "OUT", "PIECE", "DMA_PLAN")


def set_cfg(**kw):
    g = globals()
    for k, v in kw.items():
        assert k in _KNOBS, k
        g[k] = v

_cache = {}
_CUSTOM = {}


def _ensure_relu2add():
    """Register RELUADD1_GIN: out = relu(in0) + in1; accum = sum.

    in0 may be PSUM (the un-relu'd second half of a chunk); in1 is the
    already-relu'd first half in SBUF — only one PSUM operand, which is
    all the ISA allows.
    """
    if "op" in _CUSTOM:
        return _CUSTOM["op"]
    import concourse.dve_ops as dve_ops
    from concourse.dve_spec import Spec, Src0, Src1, relu, lower, _has_src1
    from concourse.dve_spec import Zero
    from concourse.dve_uop import DveOpSpec
    from operator import add

    name = "RELUADD1_GIN"

    def _ref(in0, in1, c0, c1, c2):
        b = (dve_ops._dve_relu(in0.astype(np.float32))
             + in1.astype(np.float32).reshape(in0.shape))
        return b, b.reshape(b.shape[0], -1).sum(axis=-1, keepdims=True)

    spec = Spec(body=relu(Src0) + Src1, accum=add, accum_init=Zero,
                reference=_ref)
    row = dve_ops._CUSTOM_DVE_ROW_BASE + len(dve_ops.OPS)
    assert row < 0x20
    shas = {}
    for ver in ("v3", "v4"):
        try:
            s = DveOpSpec(name=name, opcode=row, uops=lower(spec, ver=ver),
                          rd1_en=_has_src1(spec))
            shas[ver] = s.sha(ver)
        except Exception:
            pass
    op = dve_ops.DveOp(name, spec, subdim=False, uops_sha=shas)
    dve_ops.OPS.append(op)
    dve_ops.CUSTOM_DVE_SPECS[name] = spec
    dve_ops._SUB_OPCODE_FOR_NAME[name] = row
    _CUSTOM["op"] = op
    return op


def _spaces(S):
    """Derive slot-space geometry from chunk sizes."""
    S = tuple(S)
    Q = CHI * sum(S)
    cbase = [CHI * sum(S[:c]) for c in range(NCH)]
    apch = [c for g in AP_GROUPS for c in g]
    Qa = CHI * sum(S[c] for c in apch)
    apo = {}
    off = 0
    for c in apch:
        apo[c] = off
        off += CHI * S[c]
    Qd = CHI * sum(S[c] for c in DG)
    Qdp = -(-Qd // 128) * 128
    dgo = {}
    off = 0
    for c in DG:
        dgo[c] = off
        off += CHI * S[c]
    return Q, cbase, Qa, apo, Qd, Qdp, dgo


def _build_nc(mode="full", S=S_DEFAULT):
    from contextlib import ExitStack

    import concourse.mybir as mybir
    import concourse.tile as tile
    from concourse import bacc

    f32 = mybir.dt.float32
    bf16 = mybir.dt.bfloat16
    fp8 = mybir.dt.float8e4
    i16 = mybir.dt.int16
    RELU = mybir.ActivationFunctionType.Relu
    ADD = mybir.AluOpType.add
    MAX = mybir.AluOpType.max
    MULT = mybir.AluOpType.mult
    DR = mybir.MatmulPerfMode.DoubleRow

    relu2add = _ensure_relu2add()
    S = tuple(S)
    Q, cbase, Qa, apo, Qd, Qdp, dgo = _spaces(S)
    IW = Qa // 16 + Qdp // 16 + SH // 16

    nc = bacc.Bacc("TRN2", target_bir_lowering=False, debug=False,
                   enable_asserts=False, num_devices=NC)

    def din(name, shape, dt=None):
        return nc.dram_tensor(name, shape, dt or f32, kind="ExternalInput").ap()

    WC = 2 * D   # leading peT cols hold Wcomb (shared weights)
    peT_d = din("peT_sh", [KP, 2, WC + Q], fp8)
    WnB_d = din("WnB", [D, 4 * D + 64], bf16)  # Wn1T|WnT0|WnT1|row0: ones,bn1
    xb_d = din("xb_sh", [D, SH + 4])           # xown ++ (bn0 | 1+eps)
    idx_d = din("idx_sh", [128, IW], i16)
    if mode != "full":
        xe32_d = din("xe32_sh", [D, N])        # f32 allgathered-x stand-in
        xgou_d = din("xgou_sh", [N, D], bf16)  # node-major bf16 stand-in
    out_d = nc.dram_tensor("out", [SH, D], f32, kind="ExternalOutput").ap()

    with tile.TileContext(nc) as tc, ExitStack() as ctx:
        P = ctx.enter_context(tc.tile_pool(name="persist", bufs=1))
        dramp = ctx.enter_context(tc.tile_pool(name="dram", bufs=1, space="DRAM"))
        psumC = ctx.enter_context(tc.tile_pool(name="psumC", bufs=3, space="PSUM"))
        psumF = ctx.enter_context(tc.tile_pool(name="psumF", bufs=1, space="PSUM"))
        scrp = ctx.enter_context(tc.tile_pool(name="scr", bufs=3))

        # ---------------- input DMAs ----------------
        peTW = P.tile([KP, 2, WC + Q], fp8)
        peT = peTW[:, :, WC:]
        Wcomb = peTW[:, :, 0:WC]
        WnB = P.tile([D, 4 * D + 64], bf16)
        xb = P.tile([D, SH + 4], f32)
        xown = xb[:, 0:SH]
        bias = xb[:, SH:SH + 4]
        idx = P.tile([128, IW], i16)
        xe32 = P.tile([D, N], f32)

        qmap = {"S": nc.sync, "C": nc.scalar, "V": nc.vector}
        pbounds = [0, WC + cbase[1], WC + cbase[3], WC + cbase[6], WC + Q]

        def issue(q, name):
            eng = qmap[q]
            if name.startswith("peT"):
                i = int(name[3])
                lo, hi = pbounds[i], pbounds[i + 1]
                eng.dma_start(out=peTW[:, :, lo:hi], in_=peT_d[:, :, lo:hi])
            elif name == "xe32":
                if mode != "full":
                    eng.dma_start(out=xe32[:], in_=xe32_d[:])
            elif name == "WnB":
                eng.dma_start(out=WnB[:], in_=WnB_d[:])
            elif name == "xb":
                eng.dma_start(out=xb[:], in_=xb_d[:])
            elif name == "idx":
                eng.dma_start(out=idx[:], in_=idx_d[:])

        for q, name in DMA_PLAN:
            issue(q, name)

        dve_scrap = P.tile([128, 1], f32)
        Wn1T = WnB[:, 0:D]
        WnT0 = WnB[:, D:2 * D]
        WnT1 = WnB[:, 2 * D:3 * D]
        ones64 = WnB[0:1, 3 * D:3 * D + 64]
        bn1r = WnB[0:1, 3 * D + 64:4 * D + 64]
        bn0 = bias[:, 0:1]
        opse = bias[:, 1:2]

        # ---------------- exit + fold ----------------
        def exit_fold(c, ps, msg, ex, tr="D"):
            """PSUM [128, CHI*S[c]] -> relu -> segment sum -> msg cols."""
            Sc = S[c]
            W = CHI * Sc
            h = Sc // 2
            ps_r = ps[:].rearrange("p (a b) -> p a b", a=CHI)
            if ex == "H":
                # ACT relus the first half-slots; DVE fuses relu of the
                # PSUM second half with the add (one PSUM operand only).
                r1 = scrp.tile([128, CHI, h], bf16, tag=f"r1{Sc}")
                nc.scalar.activation(out=r1[:], in_=ps_r[:, :, 0:h],
                                     func=RELU)
                t1 = scrp.tile([128, CHI, h], bf16, tag=f"t1{Sc}")
                nc.vector._custom_dve(
                    relu2add, out=t1[:], in0=ps_r[:, :, h:Sc],
                    in1=r1[:], accum_out=dve_scrap[:])
            else:
                r = scrp.tile([128, CHI, Sc], bf16, tag=f"r{Sc}")
                if ex == "A":
                    nc.scalar.activation(
                        out=r[:].rearrange("p a b -> p (a b)"),
                        in_=ps[:, 0:W], func=RELU)
                elif ex == "D":
                    nc.vector.tensor_scalar(
                        out=r[:].rearrange("p a b -> p (a b)"),
                        in0=ps[:, 0:W], scalar1=0.0, scalar2=None, op0=MAX)
                else:
                    nc.gpsimd.tensor_scalar(
                        out=r[:].rearrange("p a b -> p (a b)"),
                        in0=ps[:, 0:W], scalar1=0.0, scalar2=None, op0=MAX)
                t1 = scrp.tile([128, CHI, h], bf16, tag=f"t1{Sc}")
                nc.vector.tensor_tensor(out=t1[:], in0=r[:, :, 0:h],
                                        in1=r[:, :, h:Sc], op=ADD)
            h2 = h // 2
            t2 = scrp.tile([128, CHI, h2], bf16, tag=f"t2{Sc}")
            teng = nc.gpsimd if tr == "P" else nc.vector
            teng.tensor_tensor(out=t2[:], in0=t1[:, :, 0:h2],
                               in1=t1[:, :, h2:h], op=ADD)
            reng = nc.gpsimd if tr == "Q" else nc.vector
            reng.tensor_reduce(
                out=msg[:, CHI * c:CHI * (c + 1)], in_=t2[:],
                axis=mybir.AxisListType.X, op=ADD)

        # ---------------- layer 0 ----------------
        def layer0():
            W0 = Wcomb[:, :, 0:D]
            msg = P.tile([D, SH], f32, tag="msg0")
            for c in range(NCH):
                W = CHI * S[c]
                ps = psumC.tile([128, W], f32, tag="chunk")
                for s0 in range(0, W, PIECE):
                    s1 = min(s0 + PIECE, W)
                    nc.tensor.matmul(
                        out=ps[:, s0:s1], lhsT=W0,
                        rhs=peT[:, :, cbase[c] + s0:cbase[c] + s1],
                        start=True, stop=True, perf_mode=DR)
                exit_fold(c, ps, msg, L0_EXIT[c])
            return msg

        def finish0(msg):
            h1T = P.tile([D, SH], f32, tag="h1T")
            for hh in range(2):
                sl = slice(64 * hh, 64 * (hh + 1))
                z = scrp.tile([D, 64], bf16, tag=f"z0{hh}")
                nc.vector.scalar_tensor_tensor(
                    out=z[:], in0=xown[:, sl], scalar=opse,
                    in1=msg[:, sl], op0=MULT, op1=ADD)
                ps = psumF.tile([D, 64], f32, tag="fin0")
                nc.tensor.matmul(out=ps[:], lhsT=WnT0, rhs=z[:],
                                 start=True, stop=True)
                if FIN0 == "A":
                    nc.scalar.activation(out=h1T[:, sl], in_=ps[:],
                                         func=RELU, bias=bn0)
                else:
                    nc.vector.tensor_scalar(out=h1T[:, sl], in0=ps[:],
                                            scalar1=bn0, scalar2=0.0,
                                            op0=ADD, op1=MAX)
            return h1T

        # ---------------- gathers (layer 1 node features) ----------------
        def gathers(xgou_src):
            xg1f = P.tile([128, max(Qa, 16)], f32)
            xg1b = P.tile([128, 1, max(Qdp, 128)], bf16)
            plan = []
            for gi, grp in enumerate(AP_GROUPS):
                plan.append(("ap", gi, grp))
            if DG:
                plan.insert(PREP_POS, ("dg",))
            for item in plan:
                if item[0] == "dg":
                    nc.gpsimd.dma_gather(
                        out_ap=xg1b[:, :, 0:Qdp], in_ap=xgou_src,
                        idxs_ap=idx[:, Qa // 16:Qa // 16 + Qdp // 16],
                        num_idxs=Qdp, num_idxs_reg=Qdp, elem_size=D,
                        transpose=True)
                else:
                    _, gi, grp = item
                    lo = apo[grp[0]]
                    hi = apo[grp[-1]] + CHI * S[grp[-1]]
                    nc.gpsimd.ap_gather(
                        out_ap=xg1f[:, lo:hi], in_ap=xe32[:, 0:N],
                        idxs_ap=idx[:, lo // 16:hi // 16],
                        channels=128, num_elems=N, d=1, num_idxs=hi - lo)
            return xg1f, xg1b

        # ---------------- layer 1 ----------------
        def layer1(xg1f, xg1b):
            W1e = Wcomb[0:17, :, D:2 * D]
            xgb = xg1f[:].bitcast(bf16).rearrange(
                "p (q two) -> p q two", two=2)
            msg = P.tile([D, SH], f32, tag="msg1")
            for c in L1_ORDER:
                W = CHI * S[c]
                ps = psumC.tile([128, W], f32, tag="chunk")
                for s0 in range(0, W, PIECE):
                    s1 = min(s0 + PIECE, W)
                    nc.tensor.matmul(
                        out=ps[:, s0:s1], lhsT=W1e,
                        rhs=peT[0:17, :, cbase[c] + s0:cbase[c] + s1],
                        start=True, stop=False, perf_mode=DR)
                    if c in dgo:
                        rhs = xg1b[:, 0, dgo[c] + s0:dgo[c] + s1]
                    else:
                        rhs = xgb[:, apo[c] + s0:apo[c] + s1, 1:2]
                    nc.tensor.matmul(out=ps[:, s0:s1], lhsT=Wn1T, rhs=rhs,
                                     start=False, stop=True)
                exit_fold(c, ps, msg, L1_EXIT[c], TREE1[c])
            return msg

        def finish1(msg, h1T, h2own):
            for hh in FIN1_ORDER:
                sl = slice(64 * hh, 64 * (hh + 1))
                z = scrp.tile([D, 64], bf16, tag=f"z1{hh}")
                nc.vector.scalar_tensor_tensor(
                    out=z[:], in0=h1T[:, sl], scalar=opse,
                    in1=msg[:, sl], op0=MULT, op1=ADD)
                ps = psumF.tile([64, D], f32, tag="fin1")
                nc.tensor.matmul(out=ps[:], lhsT=ones64, rhs=bn1r,
                                 start=True, stop=False)
                nc.tensor.matmul(out=ps[:], lhsT=z[:], rhs=WnT1,
                                 start=False, stop=True)
                if FIN1 == "A":
                    nc.scalar.activation(out=h2own[sl, :], in_=ps[:],
                                         func=RELU)
                else:
                    nc.vector.tensor_scalar(out=h2own[sl, :], in0=ps[:],
                                            scalar1=0.0, scalar2=None,
                                            op0=MAX)
                if OUT != "T":
                    nc.sync.dma_start(out=out_d[sl, :], in_=h2own[sl, :])
            if OUT == "T":
                nc.gpsimd.trigger_dma(count=1, queue_num=0)

        # ---------------- wiring ----------------
        h2own = P.tile([SH, D], f32)

        def out_prep():
            if OUT == "T":
                out_sem = nc.alloc_semaphore("out_dma")
                nc.gpsimd.dma_scatter_add(
                    out_d[:],
                    h2own[:].rearrange("p (g d) -> p g d", g=1),
                    idx[:, IW - SH // 16:IW],
                    SH, SH, D,
                    prepare_only=True, sem=out_sem, queue_num=0)

        if mode == "full":
            msg0 = layer0()
            h1T = finish0(msg0)
            gout = None
            if DG:
                # h1 rows (bf16, [SH, D]) for the dma_gather source
                h1Tb = P.tile([D, SH], bf16)
                nc.scalar.activation(out=h1Tb[:], in_=h1T[:],
                                     func=mybir.ActivationFunctionType.Identity)
                h1r = P.tile([SH, D], bf16)
                nc.sync.dma_start(out=h1r[:], in_=h1Tb[:], transpose=True)
                gin = dramp.tile([SH, D], bf16)
                gout = dramp.tile([N, D], bf16)
                nc.gpsimd.dma_start(out=gin[:], in_=h1r[:])
                nc.gpsimd.collective_compute(
                    "AllGather", mybir.AluOpType.bypass,
                    replica_groups=[list(range(NC))],
                    ins=[gin[:].bitcast(f32).opt()],
                    outs=[gout[:].bitcast(f32).opt()])
            if AP_GROUPS:
                gin2 = dramp.tile([D, SH], f32)
                gout2 = dramp.tile([NC * D, SH], f32)
                nc.gpsimd.dma_start(out=gin2[:], in_=h1T[:])
                nc.gpsimd.collective_compute(
                    "AllGather", mybir.AluOpType.bypass,
                    replica_groups=[list(range(NC))],
                    ins=[gin2[:].opt()], outs=[gout2[:].opt()])
                nc.sync.dma_start(
                    out=xe32[:].rearrange("p (c i) -> p c i", c=NC),
                    in_=gout2[:].rearrange("(c d) i -> d c i", d=D))
            xg1f, xg1b = gathers(gout[:] if gout is not None else None)
            out_prep()
            msg1 = layer1(xg1f, xg1b)
            finish1(msg1, h1T, h2own)
        else:
            # timed variant: no collective; gather source is a host tensor
            xg1f, xg1b = gathers(xgou_d[:] if DG else None)
            out_prep()
            msg0 = layer0()
            h1T = finish0(msg0)
            msg1 = layer1(xg1f, xg1b)
            finish1(msg1, h1T, h2own)

    nc.compile()
    return nc


def _plan(adj):
    """Degree-sort rows per core, bucket into NCH chunks, pad to mult of 4."""
    deg = adj.sum(1).astype(np.int64).reshape(NC, SH)
    perms = [np.argsort(-deg[c], kind="stable") for c in range(NC)]
    S = []
    for ch in range(NCH):
        mx = max(int(deg[c][perms[c][CHI * ch:CHI * (ch + 1)]].max())
                 for c in range(NC))
        S.append(max(4, int(-(-mx // 4) * 4)))
    return perms, tuple(S)


def _wrap_idx(L):
    """gather index layout: [128, n//16], idx[p, m] = L[m*16 + p%16]."""
    w = np.asarray(L).reshape(-1, 16).T.astype(np.int16)
    return np.tile(w, (8, 1))


def _host_inputs(inputs):
    """Build the 8 per-core input maps + plan from full inputs."""
    import ml_dtypes

    bf = ml_dtypes.bfloat16
    f8 = ml_dtypes.float8_e4m3
    adj = np.asarray(inputs["adj"], np.float32)
    nodes = np.asarray(inputs["nodes"], np.float32)
    edges = np.asarray(inputs["edges"], np.float32)
    eps = float(np.asarray(inputs["eps"], np.float32).reshape(-1)[0])
    perms, S = _plan(adj)
    Q, cbase, Qa, apo, Qd, Qdp, dgo = _spaces(S)

    # global position of node j in the allgathered (per-core sorted) layout
    gpos = np.empty(N, np.int64)
    for c in range(NC):
        gpos[c * SH + perms[c]] = c * SH + np.arange(SH)
    sorted_nodes = np.empty((N, D), np.float32)
    for c in range(NC):
        sorted_nodes[c * SH:(c + 1) * SH] = nodes[c * SH + perms[c]]

    Wne = [np.asarray(inputs["Wne0"], np.float32),
           np.asarray(inputs["Wne1"], np.float32)]
    bne = [np.asarray(inputs["bne0"], np.float32),
           np.asarray(inputs["bne1"], np.float32)]
    # Wcomb: logical contraction rows (interleaved r=2k+t):
    #   0..31 edge rows, 32 bias carrier, 33 mask, 34..161 node rows (l0)
    Wc = np.zeros((2 * KP, 2 * D), np.float32)
    for l in range(2):
        Wc[0:E, l * D:(l + 1) * D] = Wne[l][:, D:D + E].T
        Wc[E, l * D:(l + 1) * D] = bne[l]
        Wc[E + 1, l * D:(l + 1) * D] = MASKW
    Wc[E + 2:E + 2 + D, 0:D] = Wne[0][:, :D].T
    Wcomb = np.clip(Wc, -440, 440).reshape(KP, 2, 2 * D)

    WnB = np.zeros((D, 4 * D + 64), np.float32)
    WnB[:, 0:D] = Wne[1][:, :D].T
    WnB[:, D:2 * D] = np.asarray(inputs["Wn0"], np.float32).T
    WnB[:, 2 * D:3 * D] = np.asarray(inputs["Wn1"], np.float32).T
    WnB[0, 3 * D:3 * D + 64] = 1.0
    WnB[0, 3 * D + 64:4 * D + 64] = np.asarray(inputs["bn1"], np.float32)

    bias = np.zeros((D, 4), np.float32)
    bias[:, 0] = np.asarray(inputs["bn0"], np.float32)
    bias[:, 1] = 1.0 + eps

    com = {"WnB": np.ascontiguousarray(WnB.astype(bf))}

    apch = [c for g in AP_GROUPS for c in g]
    maps = []
    for c in range(NC):
        perm = perms[c]
        rows = c * SH + perm
        pe = np.zeros((Q, 2 * KP), np.float32)
        pe[:, E + 1] = -MASKW                     # mask row: pads -16
        La = np.zeros(Qa, np.int64)
        Ld = np.zeros(Qdp, np.int64)
        for p in range(SH):
            ch = p // CHI
            il = p % CHI
            base = cbase[ch] + il * S[ch]
            nbr = np.nonzero(adj[rows[p]])[0]
            k = len(nbr)
            assert k <= S[ch]
            pe[base:base + k, 0:E] = edges[rows[p], nbr]
            pe[base:base + k, E] = 1.0            # bias carrier
            pe[base:base + k, E + 1] = 0.0        # not padded
            pe[base:base + k, E + 2:E + 2 + D] = nodes[nbr]
            sbase = (dgo[ch] if ch in dgo else apo[ch]) + il * S[ch]
            tgt = Ld if ch in dgo else La
            tgt[sbase:sbase + k] = gpos[nbr]
        m = dict(com)
        peq = np.clip(pe.T, -440, 440).reshape(KP, 2, Q)
        m["peT_sh"] = np.ascontiguousarray(
            np.concatenate([Wcomb, peq], axis=2).astype(f8))
        m["xb_sh"] = np.ascontiguousarray(
            np.concatenate([nodes[rows].T, bias], axis=1))
        m["idx_sh"] = np.ascontiguousarray(
            np.concatenate([_wrap_idx(La), _wrap_idx(Ld),
                            _wrap_idx(np.arange(SH))], axis=1))
        m["xe32_sh"] = np.ascontiguousarray(sorted_nodes.T)
        m["xgou_sh"] = np.ascontiguousarray(sorted_nodes.astype(bf))
        maps.append(m)
    return maps, perms, S


def _get_runner(S):
    """Build (once per S) a cached jit(shard_map) callable."""
    key = ("runner", S)
    if key in _cache:
        return _cache[key]
    import jax
    from jax.sharding import Mesh, PartitionSpec, NamedSharding
    from jax.experimental.shard_map import shard_map
    import concourse.mybir as mybir
    from concourse import bass2jax
    from concourse.bass2jax import _bass_exec_p, partition_id_tensor

    nckey = ("nc", S)
    if nckey not in _cache:
        _cache[nckey] = _build_nc("full", S)
    nc = _cache[nckey]
    bass2jax.install_neuronx_cc_hook()

    in_names, out_names, out_avals, zero_outs = [], [], [], []
    partition_name = nc.partition_id_tensor.name if nc.partition_id_tensor else None
    for alloc in nc.m.functions[0].allocations:
        if not isinstance(alloc, mybir.MemoryLocationSet):
            continue
        name = alloc.memorylocations[0].name
        if alloc.kind == "ExternalInput":
            if name != partition_name:
                in_names.append(name)
        elif alloc.kind == "ExternalOutput":
            shape = list(alloc.tensor_shape)
            dtype = np.dtype(mybir.dt.np(alloc.dtype))
            out_avals.append(jax.core.ShapedArray(shape, dtype))
            out_names.append(name)
            zero_outs.append(np.zeros(shape, dtype))

    n_params = len(in_names)
    all_in_names = list(in_names) + list(out_names)
    if partition_name is not None:
        all_in_names.append(partition_name)

    def _body(*args):
        operands = list(args)
        if partition_name is not None:
            operands.append(partition_id_tensor())
        outs = _bass_exec_p.bind(
            *operands,
            out_avals=tuple(out_avals),
            in_names=tuple(all_in_names),
            out_names=tuple(out_names),
            lowering_input_output_aliases=(),
            sim_require_finite=True,
            sim_require_nnan=True,
            nc=nc,
        )
        return tuple(outs)

    devices = jax.devices()[:NC]
    mesh = Mesh(np.asarray(devices), ("core",))
    n_outs = len(out_names)
    fn = jax.jit(
        shard_map(_body, mesh=mesh,
                  in_specs=(PartitionSpec("core"),) * (n_params + n_outs),
                  out_specs=(PartitionSpec("core"),) * n_outs,
                  check_rep=False),
        keep_unused=True)
    sh = NamedSharding(mesh, PartitionSpec("core"))
    dev_zeros = [
        jax.device_put(np.zeros((NC * z.shape[0], *z.shape[1:]), z.dtype), sh)
        for z in zero_outs
    ]

    def run(maps):
        dev_in = []
        for nm in in_names:
            arrs = [
                jax.device_put(np.asarray(maps[c][nm]), devices[c])
                for c in range(NC)
            ]
            shp = arrs[0].shape
            glob = jax.make_array_from_single_device_arrays(
                (NC * shp[0], *shp[1:]), sh, arrs)
            dev_in.append(glob)
        outs = fn(*dev_in, *dev_zeros)
        oi = out_names.index("out")
        return np.asarray(outs[oi]).reshape(NC, SH, D)

    _cache[key] = run
    return run


def kernel(**inputs):
    maps, perms, S = _host_inputs(inputs)
    run = _get_runner(S)
    raw = run(maps)                                # [NC, SH, D], sorted rows
    out = np.empty((N, D), np.float32)
    for c in range(NC):
        out[c * SH + perms[c]] = raw[c]
    return np.ascontiguousarray(out.astype(np.float32))


if __name__ == "__main__":
    _build_nc("nocc")
    print("build+compile OK")


# revision 38
# speedup vs baseline: 1.2022x; 1.0034x over previous
"""DGINConv (2-layer GIN with edge features) Trainium2 kernel — fp8 DoubleRow.

Math (per layer, reference):
    ne[i,j,:] = relu(Wnode@x[j] + We@edges[i,j,:] + bne)
    msg[i,:]  = sum_{j: adj[i,j]=1} ne[i,j,:]
    h[i,:]    = relu(Wn @ ((1+eps)*x[i] + msg[i]) + bn)

adj density ~3%: each own row's ~31 neighbors are packed into padded slots
(host-side): own rows degree-sorted, grouped into 8 chunks of 16 rows,
chunk c padded to S_c slots/row (mult of 4). Q = 16*sum(S_c) per core.

Key structure (vs the previous bf16/identity-inject version):
- Layer 0: node term + edge term + bias + pad-mask fused into ONE fp8
  DoubleRow matmul per 256-col piece: rhs peT [81, 2, W] (logical
  contraction row r = 2k+t: 32 edge rows ++ bias carrier ++ mask(-16) ++
  128 host-gathered x rows, all fp8e4m3), lhsT Wcomb [81, 2, 128] fp8.
  DoubleRow streams 2 fp8 rows/cycle -> 0.5 PE cycles per slot (4x less
  PE time than two bf16 matmuls). Pads produce psum=-256 -> relu kills
  them; no -1e9 plumbing.
- Layer 1 reuses peT partitions 0:17 (the row interleave puts logical
  rows 0..33 = edges+bias+mask exactly there) as a [17, 2, W] fp8
  DoubleRow matmul with layer-1 weights; the node term is ap_gather'd
  per slot from an f32 copy of the allgathered features ([128, N] ->
  [128, Q]) and added via one bf16 K=128 matmul into the same PSUM
  (rhs = high-half bitcast of the gathered f32). No hbT pass, no
  identity-inject matmul, no transposes. (SWDGE dma_gather/scatter are
  NOT used: unsupported by this execution backend.)
- Exits per chunk (knob): 'A' = ACT relu [128, W] -> bf16 + DVE pairwise
  tree; 'H' = ACT relus the first half-slots, then a custom DVE op
  relu(in0)+in1 fuses the second half's relu (its single PSUM operand)
  with tree level 1. t2 tree level on DVE or Pool (TREE1), reduce DVE.
- finish l0 keeps [D, rows] (ACT relu + bn0 bias); finish l1 is emitted
  flipped (lhsT=z, rhs=WnT1, bn1 preloaded into PSUM by a K=1 matmul) so
  the output lands [rows, D] f32 and DMAs straight out.
- Weights ride in the leading 256 columns of the peT tensor (one DMA);
  bias columns ride in the xown tensor. All input DMAs issue from the
  SP queue (DMACopy holds the issuing engine's sequencer, and HWDGE is
  a serial ~630ns/DMA resource - keep it off compute engines).

Distribution: destination rows sharded 8 ways; nodes/weights replicated;
updated node features exchanged between layers via AllGather (full mode;
an f32 [D, SH] allgather feeds the gather source via one rearrange DMA).
The timed 'nocc' variant is the same program with host-fed stand-ins for
the collective outputs. Final output rows are un-permuted on the host.
"""

import sys

if "/opt/trn_rl_repo" not in sys.path:
    sys.path.insert(0, "/opt/trn_rl_repo")

import numpy as np

N, D, E, NC = 1024, 128, 32, 8
SH = N // NC          # 128 rows per core
NCH = 8               # chunks of sorted own-rows
CHI = SH // NCH       # 16 rows per chunk
KP = 81               # DoubleRow partitions: 2*81 = 162 >= 32+2+128
MASKW = 16.0          # mask weight; mask rhs = -16 -> psum -256 on pads

S_DEFAULT = (52, 40, 36, 36, 32, 32, 28, 28)

# ---------------- tuning knobs (overridable via set_cfg) ----------------
DG = ()                       # dma_gather unsupported on this backend
AP_GROUPS = ((0, 1), (2, 3, 4), (5, 6, 7))    # ap_gather call groups
PREP_POS = 0                  # index in pool sequence where dma_gather goes
L0_EXIT = "HAHAHAHA"          # A=ACT relu | D=DVE relu | H=ACT half + fused DVE
L1_EXIT = "HAHAHAHA"          # P=Pool relu (layer1 only)
TREE1 = "DDPPPPPP"            # layer1 t2 engine: D=DVE, P=Pool
L1_ORDER = (1, 0, 2, 3, 4, 5, 6, 7)   # layer1 chunk processing order
FIN0 = "A"                    # layer0 finish relu engine: A=ACT, D=DVE
FIN1 = "D"                    # layer1 finish relu engine
FIN1_ORDER = (0, 1)           # finish1 half emission order
OUT = "S"                     # output: T=SWDGE prep+trigger scatter, S=dma
PIECE = 256                   # DoubleRow piece width (<=256)
# DMA issue plan: queue S=nc.sync, C=nc.scalar, V=nc.vector; names below
DMA_PLAN = (
    ("S", "peT0"), ("S", "idx"), ("S", "peT1"), ("S", "xe32"),
    ("S", "peT2"), ("S", "peT3"), ("S", "WnB"), ("S", "xb"),
)

_KNOBS = ("DG", "AP_GROUPS", "PREP_POS", "L0_EXIT", "L1_EXIT", "TREE1",
          "L1_ORDER", "FIN0", "FIN1", "FIN1_ORDER", "OUT", "PIECE", "DMA_PLAN")


def set_cfg(**kw):
    g = globals()
    for k, v in kw.items():
        assert k in _KNOBS, k
        g[k] = v

_cache = {}
_CUSTOM = {}


def _ensure_relu2add():
    """Register RELUADD1_GIN: out = relu(in0) + in1; accum = sum.

    in0 may be PSUM (the un-relu'd second half of a chunk); in1 is the
    already-relu'd first half in SBUF — only one PSUM operand, which is
    all the ISA allows.
    """
    if "op" in _CUSTOM:
        return _CUSTOM["op"]
    import concourse.dve_ops as dve_ops
    from concourse.dve_spec import Spec, Src0, Src1, relu, lower, _has_src1
    from concourse.dve_spec import Zero
    from concourse.dve_uop import DveOpSpec
    from operator import add

    name = "RELUADD1_GIN"

    def _ref(in0, in1, c0, c1, c2):
        b = (dve_ops._dve_relu(in0.astype(np.float32))
             + in1.astype(np.float32).reshape(in0.shape))
        return b, b.reshape(b.shape[0], -1).sum(axis=-1, keepdims=True)

    spec = Spec(body=relu(Src0) + Src1, accum=add, accum_init=Zero,
                reference=_ref)
    row = dve_ops._CUSTOM_DVE_ROW_BASE + len(dve_ops.OPS)
    assert row < 0x20
    shas = {}
    for ver in ("v3", "v4"):
        try:
            s = DveOpSpec(name=name, opcode=row, uops=lower(spec, ver=ver),
                          rd1_en=_has_src1(spec))
            shas[ver] = s.sha(ver)
        except Exception:
            pass
    op = dve_ops.DveOp(name, spec, subdim=False, uops_sha=shas)
    dve_ops.OPS.append(op)
    dve_ops.CUSTOM_DVE_SPECS[name] = spec
    dve_ops._SUB_OPCODE_FOR_NAME[name] = row
    _CUSTOM["op"] = op
    return op


def _spaces(S):
    """Derive slot-space geometry from chunk sizes."""
    S = tuple(S)
    Q = CHI * sum(S)
    cbase = [CHI * sum(S[:c]) for c in range(NCH)]
    apch = [c for g in AP_GROUPS for c in g]
    Qa = CHI * sum(S[c] for c in apch)
    apo = {}
    off = 0
    for c in apch:
        apo[c] = off
        off += CHI * S[c]
    Qd = CHI * sum(S[c] for c in DG)
    Qdp = -(-Qd // 128) * 128
    dgo = {}
    off = 0
    for c in DG:
        dgo[c] = off
        off += CHI * S[c]
    return Q, cbase, Qa, apo, Qd, Qdp, dgo


def _build_nc(mode="full", S=S_DEFAULT):
    from contextlib import ExitStack

    import concourse.mybir as mybir
    import concourse.tile as tile
    from concourse import bacc

    f32 = mybir.dt.float32
    bf16 = mybir.dt.bfloat16
    fp8 = mybir.dt.float8e4
    i16 = mybir.dt.int16
    RELU = mybir.ActivationFunctionType.Relu
    ADD = mybir.AluOpType.add
    MAX = mybir.AluOpType.max
    MULT = mybir.AluOpType.mult
    DR = mybir.MatmulPerfMode.DoubleRow

    relu2add = _ensure_relu2add()
    S = tuple(S)
    Q, cbase, Qa, apo, Qd, Qdp, dgo = _spaces(S)
    IW = Qa // 16 + Qdp // 16 + SH // 16

    nc = bacc.Bacc("TRN2", target_bir_lowering=False, debug=False,
                   enable_asserts=False, num_devices=NC)

    def din(name, shape, dt=None):
        return nc.dram_tensor(name, shape, dt or f32, kind="ExternalInput").ap()

    WC = 2 * D   # leading peT cols hold Wcomb (shared weights)
    peT_d = din("peT_sh", [KP, 2, WC + Q], fp8)
    WnB_d = din("WnB", [D, 4 * D + 64], bf16)  # Wn1T|WnT0|WnT1|row0: ones,bn1
    xb_d = din("xb_sh", [D, SH + 4])           # xown ++ (bn0 | 1+eps)
    idx_d = din("idx_sh", [128, IW], i16)
    if mode != "full":
        xe32_d = din("xe32_sh", [D, N])        # f32 allgathered-x stand-in
        xgou_d = din("xgou_sh", [N, D], bf16)  # node-major bf16 stand-in
    out_d = nc.dram_tensor("out", [SH, D], f32, kind="ExternalOutput").ap()

    with tile.TileContext(nc) as tc, ExitStack() as ctx:
        P = ctx.enter_context(tc.tile_pool(name="persist", bufs=1))
        dramp = ctx.enter_context(tc.tile_pool(name="dram", bufs=1, space="DRAM"))
        psumC = ctx.enter_context(tc.tile_pool(name="psumC", bufs=3, space="PSUM"))
        psumF = ctx.enter_context(tc.tile_pool(name="psumF", bufs=1, space="PSUM"))
        scrp = ctx.enter_context(tc.tile_pool(name="scr", bufs=3))

        # ---------------- input DMAs ----------------
        peTW = P.tile([KP, 2, WC + Q], fp8)
        peT = peTW[:, :, WC:]
        Wcomb = peTW[:, :, 0:WC]
        WnB = P.tile([D, 4 * D + 64], bf16)
        xb = P.tile([D, SH + 4], f32)
        xown = xb[:, 0:SH]
        bias = xb[:, SH:SH + 4]
        idx = P.tile([128, IW], i16)
        xe32 = P.tile([D, N], f32)

        qmap = {"S": nc.sync, "C": nc.scalar, "V": nc.vector}
        pbounds = [0, WC + cbase[1], WC + cbase[3], WC + cbase[6], WC + Q]

        def issue(q, name):
            eng = qmap[q]
            if name.startswith("peT"):
                i = int(name[3])
                lo, hi = pbounds[i], pbounds[i + 1]
                eng.dma_start(out=peTW[:, :, lo:hi], in_=peT_d[:, :, lo:hi])
            elif name == "xe32":
                if mode != "full":
                    eng.dma_start(out=xe32[:], in_=xe32_d[:])
            elif name == "WnB":
                eng.dma_start(out=WnB[:], in_=WnB_d[:])
            elif name == "xb":
                eng.dma_start(out=xb[:], in_=xb_d[:])
            elif name == "idx":
                eng.dma_start(out=idx[:], in_=idx_d[:])

        for q, name in DMA_PLAN:
            issue(q, name)

        dve_scrap = P.tile([128, 1], f32)
        Wn1T = WnB[:, 0:D]
        WnT0 = WnB[:, D:2 * D]
        WnT1 = WnB[:, 2 * D:3 * D]
        ones64 = WnB[0:1, 3 * D:3 * D + 64]
        bn1r = WnB[0:1, 3 * D + 64:4 * D + 64]
        bn0 = bias[:, 0:1]
        opse = bias[:, 1:2]

        # ---------------- exit + fold ----------------
        def exit_fold(c, ps, msg, ex, tr="D"):
            """PSUM [128, CHI*S[c]] -> relu -> segment sum -> msg cols."""
            Sc = S[c]
            W = CHI * Sc
            h = Sc // 2
            ps_r = ps[:].rearrange("p (a b) -> p a b", a=CHI)
            if ex == "H":
                # ACT relus the first half-slots; DVE fuses relu of the
                # PSUM second half with the add (one PSUM operand only).
                r1 = scrp.tile([128, CHI, h], bf16, tag=f"r1{Sc}")
                nc.scalar.activation(out=r1[:], in_=ps_r[:, :, 0:h],
                                     func=RELU)
                t1 = scrp.tile([128, CHI, h], bf16, tag=f"t1{Sc}")
                nc.vector._custom_dve(
                    relu2add, out=t1[:], in0=ps_r[:, :, h:Sc],
                    in1=r1[:], accum_out=dve_scrap[:])
            else:
                r = scrp.tile([128, CHI, Sc], bf16, tag=f"r{Sc}")
                if ex == "A":
                    nc.scalar.activation(
                        out=r[:].rearrange("p a b -> p (a b)"),
                        in_=ps[:, 0:W], func=RELU)
                elif ex == "D":
                    nc.vector.tensor_scalar(
                        out=r[:].rearrange("p a b -> p (a b)"),
                        in0=ps[:, 0:W], scalar1=0.0, scalar2=None, op0=MAX)
                else:
                    nc.gpsimd.tensor_scalar(
                        out=r[:].rearrange("p a b -> p (a b)"),
                        in0=ps[:, 0:W], scalar1=0.0, scalar2=None, op0=MAX)
                t1 = scrp.tile([128, CHI, h], bf16, tag=f"t1{Sc}")
                nc.vector.tensor_tensor(out=t1[:], in0=r[:, :, 0:h],
                                        in1=r[:, :, h:Sc], op=ADD)
            h2 = h // 2
            t2 = scrp.tile([128, CHI, h2], bf16, tag=f"t2{Sc}")
            teng = nc.gpsimd if tr == "P" else nc.vector
            teng.tensor_tensor(out=t2[:], in0=t1[:, :, 0:h2],
                               in1=t1[:, :, h2:h], op=ADD)
            reng = nc.gpsimd if tr == "Q" else nc.vector
            reng.tensor_reduce(
                out=msg[:, CHI * c:CHI * (c + 1)], in_=t2[:],
                axis=mybir.AxisListType.X, op=ADD)

        # ---------------- layer 0 ----------------
        def layer0():
            W0 = Wcomb[:, :, 0:D]
            msg = P.tile([D, SH], f32, tag="msg0")
            for c in range(NCH):
                W = CHI * S[c]
                ps = psumC.tile([128, W], f32, tag="chunk")
                for s0 in range(0, W, PIECE):
                    s1 = min(s0 + PIECE, W)
                    nc.tensor.matmul(
                        out=ps[:, s0:s1], lhsT=W0,
                        rhs=peT[:, :, cbase[c] + s0:cbase[c] + s1],
                        start=True, stop=True, perf_mode=DR)
                exit_fold(c, ps, msg, L0_EXIT[c])
            return msg

        def finish0(msg):
            h1T = P.tile([D, SH], f32, tag="h1T")
            for hh in range(2):
                sl = slice(64 * hh, 64 * (hh + 1))
                z = scrp.tile([D, 64], bf16, tag=f"z0{hh}")
                nc.vector.scalar_tensor_tensor(
                    out=z[:], in0=xown[:, sl], scalar=opse,
                    in1=msg[:, sl], op0=MULT, op1=ADD)
                ps = psumF.tile([D, 64], f32, tag="fin0")
                nc.tensor.matmul(out=ps[:], lhsT=WnT0, rhs=z[:],
                                 start=True, stop=True)
                if FIN0 == "A":
                    nc.scalar.activation(out=h1T[:, sl], in_=ps[:],
                                         func=RELU, bias=bn0)
                else:
                    nc.vector.tensor_scalar(out=h1T[:, sl], in0=ps[:],
                                            scalar1=bn0, scalar2=0.0,
                                            op0=ADD, op1=MAX)
            return h1T

        # ---------------- gathers (layer 1 node features) ----------------
        def gathers(xgou_src):
            xg1f = P.tile([128, max(Qa, 16)], f32)
            xg1b = P.tile([128, 1, max(Qdp, 128)], bf16)
            plan = []
            for gi, grp in enumerate(AP_GROUPS):
                plan.append(("ap", gi, grp))
            if DG:
                plan.insert(PREP_POS, ("dg",))
            for item in plan:
                if item[0] == "dg":
                    nc.gpsimd.dma_gather(
                        out_ap=xg1b[:, :, 0:Qdp], in_ap=xgou_src,
                        idxs_ap=idx[:, Qa // 16:Qa // 16 + Qdp // 16],
                        num_idxs=Qdp, num_idxs_reg=Qdp, elem_size=D,
                        transpose=True)
                else:
                    _, gi, grp = item
                    lo = apo[grp[0]]
                    hi = apo[grp[-1]] + CHI * S[grp[-1]]
                    nc.gpsimd.ap_gather(
                        out_ap=xg1f[:, lo:hi], in_ap=xe32[:, 0:N],
                        idxs_ap=idx[:, lo // 16:hi // 16],
                        channels=128, num_elems=N, d=1, num_idxs=hi - lo)
            return xg1f, xg1b

        # ---------------- layer 1 ----------------
        def layer1(xg1f, xg1b):
            W1e = Wcomb[0:17, :, D:2 * D]
            xgb = xg1f[:].bitcast(bf16).rearrange(
                "p (q two) -> p q two", two=2)
            msg = P.tile([D, SH], f32, tag="msg1")
            for c in L1_ORDER:
                W = CHI * S[c]
                ps = psumC.tile([128, W], f32, tag="chunk")
                for s0 in range(0, W, PIECE):
                    s1 = min(s0 + PIECE, W)
                    nc.tensor.matmul(
                        out=ps[:, s0:s1], lhsT=W1e,
                        rhs=peT[0:17, :, cbase[c] + s0:cbase[c] + s1],
                        start=True, stop=False, perf_mode=DR)
                    if c in dgo:
                        rhs = xg1b[:, 0, dgo[c] + s0:dgo[c] + s1]
                    else:
                        rhs = xgb[:, apo[c] + s0:apo[c] + s1, 1:2]
                    nc.tensor.matmul(out=ps[:, s0:s1], lhsT=Wn1T, rhs=rhs,
                                     start=False, stop=True)
                exit_fold(c, ps, msg, L1_EXIT[c], TREE1[c])
            return msg

        def finish1(msg, h1T, h2own):
            for hh in FIN1_ORDER:
                sl = slice(64 * hh, 64 * (hh + 1))
                z = scrp.tile([D, 64], bf16, tag=f"z1{hh}")
                nc.vector.scalar_tensor_tensor(
                    out=z[:], in0=h1T[:, sl], scalar=opse,
                    in1=msg[:, sl], op0=MULT, op1=ADD)
                ps = psumF.tile([64, D], f32, tag="fin1")
                nc.tensor.matmul(out=ps[:], lhsT=ones64, rhs=bn1r,
                                 start=True, stop=False)
                nc.tensor.matmul(out=ps[:], lhsT=z[:], rhs=WnT1,
                                 start=False, stop=True)
                if FIN1 == "A":
                    nc.scalar.activation(out=h2own[sl, :], in_=ps[:],
                                         func=RELU)
                else:
                    nc.vector.tensor_scalar(out=h2own[sl, :], in0=ps[:],
                                            scalar1=0.0, scalar2=None,
                                            op0=MAX)
                if OUT != "T":
                    nc.sync.dma_start(out=out_d[sl, :], in_=h2own[sl, :])
            if OUT == "T":
                nc.gpsimd.trigger_dma(count=1, queue_num=0)

        # ---------------- wiring ----------------
        h2own = P.tile([SH, D], f32)

        def out_prep():
            if OUT == "T":
                out_sem = nc.alloc_semaphore("out_dma")
                nc.gpsimd.dma_scatter_add(
                    out_d[:],
                    h2own[:].rearrange("p (g d) -> p g d", g=1),
                    idx[:, IW - SH // 16:IW],
                    SH, SH, D,
                    prepare_only=True, sem=out_sem, queue_num=0)

        if mode == "full":
            msg0 = layer0()
            h1T = finish0(msg0)
            gout = None
            if DG:
                # h1 rows (bf16, [SH, D]) for the dma_gather source
                h1Tb = P.tile([D, SH], bf16)
                nc.scalar.activation(out=h1Tb[:], in_=h1T[:],
                                     func=mybir.ActivationFunctionType.Identity)
                h1r = P.tile([SH, D], bf16)
                nc.sync.dma_start(out=h1r[:], in_=h1Tb[:], transpose=True)
                gin = dramp.tile([SH, D], bf16)
                gout = dramp.tile([N, D], bf16)
                nc.gpsimd.dma_start(out=gin[:], in_=h1r[:])
                nc.gpsimd.collective_compute(
                    "AllGather", mybir.AluOpType.bypass,
                    replica_groups=[list(range(NC))],
                    ins=[gin[:].bitcast(f32).opt()],
                    outs=[gout[:].bitcast(f32).opt()])
            if AP_GROUPS:
                gin2 = dramp.tile([D, SH], f32)
                gout2 = dramp.tile([NC * D, SH], f32)
                nc.gpsimd.dma_start(out=gin2[:], in_=h1T[:])
                nc.gpsimd.collective_compute(
                    "AllGather", mybir.AluOpType.bypass,
                    replica_groups=[list(range(NC))],
                    ins=[gin2[:].opt()], outs=[gout2[:].opt()])
                nc.sync.dma_start(
                    out=xe32[:].rearrange("p (c i) -> p c i", c=NC),
                    in_=gout2[:].rearrange("(c d) i -> d c i", d=D))
            xg1f, xg1b = gathers(gout[:] if gout is not None else None)
            out_prep()
            msg1 = layer1(xg1f, xg1b)
            finish1(msg1, h1T, h2own)
        else:
            # timed variant: no collective; gather source is a host tensor
            xg1f, xg1b = gathers(xgou_d[:] if DG else None)
            out_prep()
            msg0 = layer0()
            h1T = finish0(msg0)
            msg1 = layer1(xg1f, xg1b)
            finish1(msg1, h1T, h2own)

    nc.compile()
    return nc


def _plan(adj):
    """Degree-sort rows per core, bucket into NCH chunks, pad to mult of 4."""
    deg = adj.sum(1).astype(np.int64).reshape(NC, SH)
    perms = [np.argsort(-deg[c], kind="stable") for c in range(NC)]
    S = []
    for ch in range(NCH):
        mx = max(int(deg[c][perms[c][CHI * ch:CHI * (ch + 1)]].max())
                 for c in range(NC))
        S.append(max(4, int(-(-mx // 4) * 4)))
    return perms, tuple(S)


def _wrap_idx(L):
    """gather index layout: [128, n//16], idx[p, m] = L[m*16 + p%16]."""
    w = np.asarray(L).reshape(-1, 16).T.astype(np.int16)
    return np.tile(w, (8, 1))


def _host_inputs(inputs):
    """Build the 8 per-core input maps + plan from full inputs."""
    import ml_dtypes

    bf = ml_dtypes.bfloat16
    f8 = ml_dtypes.float8_e4m3
    adj = np.asarray(inputs["adj"], np.float32)
    nodes = np.asarray(inputs["nodes"], np.float32)
    edges = np.asarray(inputs["edges"], np.float32)
    eps = float(np.asarray(inputs["eps"], np.float32).reshape(-1)[0])
    perms, S = _plan(adj)
    Q, cbase, Qa, apo, Qd, Qdp, dgo = _spaces(S)

    # global position of node j in the allgathered (per-core sorted) layout
    gpos = np.empty(N, np.int64)
    for c in range(NC):
        gpos[c * SH + perms[c]] = c * SH + np.arange(SH)
    sorted_nodes = np.empty((N, D), np.float32)
    for c in range(NC):
        sorted_nodes[c * SH:(c + 1) * SH] = nodes[c * SH + perms[c]]

    Wne = [np.asarray(inputs["Wne0"], np.float32),
           np.asarray(inputs["Wne1"], np.float32)]
    bne = [np.asarray(inputs["bne0"], np.float32),
           np.asarray(inputs["bne1"], np.float32)]
    # Wcomb: logical contraction rows (interleaved r=2k+t):
    #   0..31 edge rows, 32 bias carrier, 33 mask, 34..161 node rows (l0)
    Wc = np.zeros((2 * KP, 2 * D), np.float32)
    for l in range(2):
        Wc[0:E, l * D:(l + 1) * D] = Wne[l][:, D:D + E].T
        Wc[E, l * D:(l + 1) * D] = bne[l]
        Wc[E + 1, l * D:(l + 1) * D] = MASKW
    Wc[E + 2:E + 2 + D, 0:D] = Wne[0][:, :D].T
    Wcomb = np.clip(Wc, -440, 440).reshape(KP, 2, 2 * D)

    WnB = np.zeros((D, 4 * D + 64), np.float32)
    WnB[:, 0:D] = Wne[1][:, :D].T
    WnB[:, D:2 * D] = np.asarray(inputs["Wn0"], np.float32).T
    WnB[:, 2 * D:3 * D] = np.asarray(inputs["Wn1"], np.float32).T
    WnB[0, 3 * D:3 * D + 64] = 1.0
    WnB[0, 3 * D + 64:4 * D + 64] = np.asarray(inputs["bn1"], np.float32)

    bias = np.zeros((D, 4), np.float32)
    bias[:, 0] = np.asarray(inputs["bn0"], np.float32)
    bias[:, 1] = 1.0 + eps

    com = {"WnB": np.ascontiguousarray(WnB.astype(bf))}

    apch = [c for g in AP_GROUPS for c in g]
    maps = []
    for c in range(NC):
        perm = perms[c]
        rows = c * SH + perm
        pe = np.zeros((Q, 2 * KP), np.float32)
        pe[:, E + 1] = -MASKW                     # mask row: pads -16
        La = np.zeros(Qa, np.int64)
        Ld = np.zeros(Qdp, np.int64)
        for p in range(SH):
            ch = p // CHI
            il = p % CHI
            base = cbase[ch] + il * S[ch]
            nbr = np.nonzero(adj[rows[p]])[0]
            k = len(nbr)
            assert k <= S[ch]
            pe[base:base + k, 0:E] = edges[rows[p], nbr]
            pe[base:base + k, E] = 1.0            # bias carrier
            pe[base:base + k, E + 1] = 0.0        # not padded
            pe[base:base + k, E + 2:E + 2 + D] = nodes[nbr]
            sbase = (dgo[ch] if ch in dgo else apo[ch]) + il * S[ch]
            tgt = Ld if ch in dgo else La
            tgt[sbase:sbase + k] = gpos[nbr]
        m = dict(com)
        peq = np.clip(pe.T, -440, 440).reshape(KP, 2, Q)
        m["peT_sh"] = np.ascontiguousarray(
            np.concatenate([Wcomb, peq], axis=2).astype(f8))
        m["xb_sh"] = np.ascontiguousarray(
            np.concatenate([nodes[rows].T, bias], axis=1))
        m["idx_sh"] = np.ascontiguousarray(
            np.concatenate([_wrap_idx(La), _wrap_idx(Ld),
                            _wrap_idx(np.arange(SH))], axis=1))
        m["xe32_sh"] = np.ascontiguousarray(sorted_nodes.T)
        m["xgou_sh"] = np.ascontiguousarray(sorted_nodes.astype(bf))
        maps.append(m)
    return maps, perms, S


def _get_runner(S):
    """Build (once per S) a cached jit(shard_map) callable."""
    key = ("runner", S)
    if key in _cache:
        return _cache[key]
    import jax
    from jax.sharding import Mesh, PartitionSpec, NamedSharding
    from jax.experimental.shard_map import shard_map
    import concourse.mybir as mybir
    from concourse import bass2jax
    from concourse.bass2jax import _bass_exec_p, partition_id_tensor

    nckey = ("nc", S)
    if nckey not in _cache:
        _cache[nckey] = _build_nc("full", S)
    nc = _cache[nckey]
    bass2jax.install_neuronx_cc_hook()

    in_names, out_names, out_avals, zero_outs = [], [], [], []
    partition_name = nc.partition_id_tensor.name if nc.partition_id_tensor else None
    for alloc in nc.m.functions[0].allocations:
        if not isinstance(alloc, mybir.MemoryLocationSet):
            continue
        name = alloc.memorylocations[0].name
        if alloc.kind == "ExternalInput":
            if name != partition_name:
                in_names.append(name)
        elif alloc.kind == "ExternalOutput":
            shape = list(alloc.tensor_shape)
            dtype = np.dtype(mybir.dt.np(alloc.dtype))
            out_avals.append(jax.core.ShapedArray(shape, dtype))
            out_names.append(name)
            zero_outs.append(np.zeros(shape, dtype))

    n_params = len(in_names)
    all_in_names = list(in_names) + list(out_names)
    if partition_name is not None:
        all_in_names.append(partition_name)

    def _body(*args):
        operands = list(args)
        if partition_name is not None:
            operands.append(partition_id_tensor())
        outs = _bass_exec_p.bind(
            *operands,
            out_avals=tuple(out_avals),
            in_names=tuple(all_in_names),
            out_names=tuple(out_names),
            lowering_input_output_aliases=(),
            sim_require_finite=True,
            sim_require_nnan=True,
            nc=nc,
        )
        return tuple(outs)

    devices = jax.devices()[:NC]
    mesh = Mesh(np.asarray(devices), ("core",))
    n_outs = len(out_names)
    fn = jax.jit(
        shard_map(_body, mesh=mesh,
                  in_specs=(PartitionSpec("core"),) * (n_params + n_outs),
                  out_specs=(PartitionSpec("core"),) * n_outs,
                  check_rep=False),
        keep_unused=True)
    sh = NamedSharding(mesh, PartitionSpec("core"))
    dev_zeros = [
        jax.device_put(np.zeros((NC * z.shape[0], *z.shape[1:]), z.dtype), sh)
        for z in zero_outs
    ]

    def run(maps):
        dev_in = []
        for nm in in_names:
            arrs = [
                jax.device_put(np.asarray(maps[c][nm]), devices[c])
                for c in range(NC)
            ]
            shp = arrs[0].shape
            glob = jax.make_array_from_single_device_arrays(
                (NC * shp[0], *shp[1:]), sh, arrs)
            dev_in.append(glob)
        outs = fn(*dev_in, *dev_zeros)
        oi = out_names.index("out")
        return np.asarray(outs[oi]).reshape(NC, SH, D)

    _cache[key] = run
    return run


def kernel(**inputs):
    maps, perms, S = _host_inputs(inputs)
    run = _get_runner(S)
    raw = run(maps)                                # [NC, SH, D], sorted rows
    out = np.empty((N, D), np.float32)
    for c in range(NC):
        out[c * SH + perms[c]] = raw[c]
    return np.ascontiguousarray(out.astype(np.float32))


if __name__ == "__main__":
    _build_nc("nocc")
    print("build+compile OK")


# revision 54
# speedup vs baseline: 1.2917x; 1.0745x over previous
"""DGINConv (2-layer GIN with edge features) Trainium2 kernel — fp8 DoubleRow.

Math (per layer, reference):
    ne[i,j,:] = relu(Wnode@x[j] + We@edges[i,j,:] + bne)
    msg[i,:]  = sum_{j: adj[i,j]=1} ne[i,j,:]
    h[i,:]    = relu(Wn @ ((1+eps)*x[i] + msg[i]) + bn)

adj density ~3%: each own row's ~31 neighbors are packed into padded slots
(host-side): own rows degree-sorted, grouped into 8 chunks of 16 rows,
chunk c padded to S_c slots/row (mult of 4). Q = 16*sum(S_c) per core.

Key structure (vs the previous bf16/identity-inject version):
- Layer 0: node term + edge term + bias + pad-mask fused into ONE fp8
  DoubleRow matmul per 256-col piece: rhs peT [81, 2, W] (logical
  contraction row r = 2k+t: 32 edge rows ++ bias carrier ++ mask(-16) ++
  128 host-gathered x rows, all fp8e4m3), lhsT Wcomb [81, 2, 128] fp8.
  DoubleRow streams 2 fp8 rows/cycle -> 0.5 PE cycles per slot (4x less
  PE time than two bf16 matmuls). Pads produce psum=-256 -> relu kills
  them; no -1e9 plumbing.
- Layer 1 reuses peT partitions 0:17 (the row interleave puts logical
  rows 0..33 = edges+bias+mask exactly there) as a [17, 2, W] fp8
  DoubleRow matmul with layer-1 weights; the node term is ap_gather'd
  per slot from an f32 copy of the allgathered features ([128, N] ->
  [128, Q]) and added via one bf16 K=128 matmul into the same PSUM
  (rhs = high-half bitcast of the gathered f32). No hbT pass, no
  identity-inject matmul, no transposes. (SWDGE dma_gather/scatter are
  NOT used: unsupported by this execution backend.)
- Exits per chunk (knob): 'A' = ACT relu [128, W] -> bf16 + DVE pairwise
  tree; 'H' = ACT relus the first half-slots, then a custom DVE op
  relu(in0)+in1 fuses the second half's relu (its single PSUM operand)
  with tree level 1. t2 tree level on DVE or Pool (TREE1), reduce DVE.
- finishes exploit linearity: Wn((1+eps)x + msg) = Wn(1+eps)x + Wn msg,
  so no z staging pass - two accumulating matmuls per half against a
  host-scaled bf16 x copy and a high-half-bitcast bf16 view of the f32
  msg tile. finish l0 keeps [D, rows] (ACT relu + bn0 bias); finish l1
  is emitted flipped (stationary = h1/msg columns, moving = WnT1, bn1
  preloaded into PSUM by a K=1 matmul) so the output lands [rows, D]
  f32 and DMAs straight out.
- Weights ride in the leading 256 columns of the peT tensor (one DMA);
  bias columns ride in the xown tensor. All input DMAs issue from the
  SP queue (DMACopy holds the issuing engine's sequencer, and HWDGE is
  a serial ~630ns/DMA resource - keep it off compute engines).

Distribution: destination rows sharded 8 ways; nodes/weights replicated;
updated node features exchanged between layers via AllGather (full mode;
an f32 [D, SH] allgather feeds the gather source via one rearrange DMA).
The timed 'nocc' variant is the same program with host-fed stand-ins for
the collective outputs. Final output rows are un-permuted on the host.
"""

import sys

if "/opt/trn_rl_repo" not in sys.path:
    sys.path.insert(0, "/opt/trn_rl_repo")

import numpy as np

N, D, E, NC = 1024, 128, 32, 8
SH = N // NC          # 128 rows per core
NCH = 8               # chunks of sorted own-rows
CHI = SH // NCH       # 16 rows per chunk
KP = 81               # DoubleRow partitions: 2*81 = 162 >= 32+2+128
MASKW = 16.0          # mask weight; mask rhs = -16 -> psum -256 on pads

S_DEFAULT = (52, 40, 36, 32, 32, 32, 28, 28)

# ---------------- tuning knobs (overridable via set_cfg) ----------------
DG = ()                       # dma_gather unsupported on this backend
AP_GROUPS = ((0, 1), (2, 3, 4), (5, 6, 7))    # ap_gather call groups
PREP_POS = 0                  # index in pool sequence where dma_gather goes
L0_EXIT = "HAHAHAHA"          # A=ACT relu | D=DVE relu | H=ACT half + fused DVE
L1_EXIT = "HAAHAAHA"          # P=Pool relu (layer1 only)
TREE1 = "DDPPPPPP"            # layer1 t2 engine: D=DVE, P=Pool
L1_ORDER = (0, 1, 2, 3, 4, 5, 6, 7)   # layer1 chunk processing order
FIN0 = "A"                    # layer0 finish relu engine: A=ACT, D=DVE
FIN1 = "DD"                   # layer1 finish relu engine per half
FIN1_ORDER = (0, 1)           # finish1 half emission order
OUT = "S"                     # output: T=SWDGE prep+trigger scatter, S=dma
PET_SPLITS = (3, 4)           # peT piece boundaries (cbase indices)
PIECE = 256                   # DoubleRow piece width (<=256)
# DMA issue plan: queue S=nc.sync, C=nc.scalar, V=nc.vector; names below
DMA_PLAN = (
    ("S", "peT0"), ("S", "idx"), ("S", "xe32"),
    ("S", "peT1"), ("S", "peT2"), ("S", "WnB"), ("S", "xb"),
)

_KNOBS = ("DG", "AP_GROUPS", "PREP_POS", "L0_EXIT", "L1_EXIT", "TREE1",
          "L1_ORDER", "FIN0", "FIN1", "FIN1_ORDER", "OUT", "PIECE", "DMA_PLAN",
          "PET_SPLITS")


def set_cfg(**kw):
    g = globals()
    for k, v in kw.items():
        assert k in _KNOBS, k
        g[k] = v

_cache = {}
_CUSTOM = {}


def _ensure_relu2add():
    """Register RELUADD1_GIN: out = relu(in0) + in1; accum = sum.

    in0 may be PSUM (the un-relu'd second half of a chunk); in1 is the
    already-relu'd first half in SBUF — only one PSUM operand, which is
    all the ISA allows.
    """
    if "op" in _CUSTOM:
        return _CUSTOM["op"]
    import concourse.dve_ops as dve_ops
    from concourse.dve_spec import Spec, Src0, Src1, relu, lower, _has_src1
    from concourse.dve_spec import Zero
    from concourse.dve_uop import DveOpSpec
    from operator import add

    name = "RELUADD1_GIN"

    def _ref(in0, in1, c0, c1, c2):
        b = (dve_ops._dve_relu(in0.astype(np.float32))
             + in1.astype(np.float32).reshape(in0.shape))
        return b, b.reshape(b.shape[0], -1).sum(axis=-1, keepdims=True)

    spec = Spec(body=relu(Src0) + Src1, accum=add, accum_init=Zero,
                reference=_ref)
    row = dve_ops._CUSTOM_DVE_ROW_BASE + len(dve_ops.OPS)
    assert row < 0x20
    shas = {}
    for ver in ("v3", "v4"):
        try:
            s = DveOpSpec(name=name, opcode=row, uops=lower(spec, ver=ver),
                          rd1_en=_has_src1(spec))
            shas[ver] = s.sha(ver)
        except Exception:
            pass
    op = dve_ops.DveOp(name, spec, subdim=False, uops_sha=shas)
    dve_ops.OPS.append(op)
    dve_ops.CUSTOM_DVE_SPECS[name] = spec
    dve_ops._SUB_OPCODE_FOR_NAME[name] = row
    _CUSTOM["op"] = op
    return op


def _spaces(S):
    """Derive slot-space geometry from chunk sizes."""
    S = tuple(S)
    Q = CHI * sum(S)
    cbase = [CHI * sum(S[:c]) for c in range(NCH)]
    apch = [c for g in AP_GROUPS for c in g]
    Qa = CHI * sum(S[c] for c in apch)
    apo = {}
    off = 0
    for c in apch:
        apo[c] = off
        off += CHI * S[c]
    Qd = CHI * sum(S[c] for c in DG)
    Qdp = -(-Qd // 128) * 128
    dgo = {}
    off = 0
    for c in DG:
        dgo[c] = off
        off += CHI * S[c]
    return Q, cbase, Qa, apo, Qd, Qdp, dgo


def _build_nc(mode="full", S=S_DEFAULT):
    from contextlib import ExitStack

    import concourse.mybir as mybir
    import concourse.tile as tile
    from concourse import bacc

    f32 = mybir.dt.float32
    bf16 = mybir.dt.bfloat16
    fp8 = mybir.dt.float8e4
    i16 = mybir.dt.int16
    RELU = mybir.ActivationFunctionType.Relu
    ADD = mybir.AluOpType.add
    MAX = mybir.AluOpType.max
    MULT = mybir.AluOpType.mult
    DR = mybir.MatmulPerfMode.DoubleRow

    relu2add = _ensure_relu2add()
    S = tuple(S)
    Q, cbase, Qa, apo, Qd, Qdp, dgo = _spaces(S)
    IW = Qa // 16 + Qdp // 16 + SH // 16

    nc = bacc.Bacc("TRN2", target_bir_lowering=False, debug=False,
                   enable_asserts=False, num_devices=NC)

    def din(name, shape, dt=None):
        return nc.dram_tensor(name, shape, dt or f32, kind="ExternalInput").ap()

    WC = 2 * D   # leading peT cols hold Wcomb (shared weights)
    peT_d = din("peT_sh", [KP, 2, WC + Q], fp8)
    WnB_d = din("WnB", [D, 6 * D + 64], bf16)  # Wn1T|WnT0|WnT1|row0:ones,bn1|xs|WnT1s
    xb_d = din("xb_sh", [D, SH + 4])           # xown ++ (bn0 | 1+eps)
    idx_d = din("idx_sh", [128, IW], i16)
    if mode != "full":
        xe32_d = din("xe32_sh", [D, N])        # f32 allgathered-x stand-in
        xgou_d = din("xgou_sh", [N, D], bf16)  # node-major bf16 stand-in
    out_d = nc.dram_tensor("out", [SH, D], f32, kind="ExternalOutput").ap()

    with tile.TileContext(nc) as tc, ExitStack() as ctx:
        P = ctx.enter_context(tc.tile_pool(name="persist", bufs=1))
        dramp = ctx.enter_context(tc.tile_pool(name="dram", bufs=1, space="DRAM"))
        psumC = ctx.enter_context(tc.tile_pool(name="psumC", bufs=3, space="PSUM"))
        psumF = ctx.enter_context(tc.tile_pool(name="psumF", bufs=1, space="PSUM"))
        scrp = ctx.enter_context(tc.tile_pool(name="scr", bufs=3))

        # ---------------- input DMAs ----------------
        peTW = P.tile([KP, 2, WC + Q], fp8)
        peT = peTW[:, :, WC:]
        Wcomb = peTW[:, :, 0:WC]
        WnB = P.tile([D, 6 * D + 64], bf16)
        xb = P.tile([D, SH + 4], f32)
        xown = xb[:, 0:SH]
        bias = xb[:, SH:SH + 4]
        idx = P.tile([128, IW], i16)
        xe32 = P.tile([D, N], f32)

        qmap = {"S": nc.sync, "C": nc.scalar, "V": nc.vector}
        pbounds = ([0] + [WC + cbase[i] for i in PET_SPLITS]
                   + [WC + Q])

        def issue(q, name):
            eng = qmap[q]
            if name.startswith("peT"):
                i = int(name[3])
                lo, hi = pbounds[i], pbounds[i + 1]
                eng.dma_start(out=peTW[:, :, lo:hi], in_=peT_d[:, :, lo:hi])
            elif name == "xe32":
                if mode != "full":
                    eng.dma_start(out=xe32[:], in_=xe32_d[:])
            elif name == "WnB":
                eng.dma_start(out=WnB[:], in_=WnB_d[:])
            elif name == "xb":
                eng.dma_start(out=xb[:], in_=xb_d[:])
            elif name == "idx":
                eng.dma_start(out=idx[:], in_=idx_d[:])

        for q, name in DMA_PLAN:
            issue(q, name)

        dve_scrap = P.tile([128, 1], f32)
        Wn1T = WnB[:, 0:D]
        WnT0 = WnB[:, D:2 * D]
        WnT1 = WnB[:, 2 * D:3 * D]
        ones64 = WnB[0:1, 3 * D:3 * D + 64]
        bn1r = WnB[0:1, 3 * D + 64:4 * D + 64]
        xs_bf = WnB[:, 4 * D + 64:5 * D + 64]   # (1+eps)*xown bf16
        WnT1s = WnB[:, 5 * D + 64:6 * D + 64]   # (1+eps)*Wn1.T bf16
        bn0 = bias[:, 0:1]
        opse = bias[:, 1:2]

        # ---------------- exit + fold ----------------
        def exit_fold(c, ps, msg, ex, tr="D"):
            """PSUM [128, CHI*S[c]] -> relu -> segment sum -> msg cols."""
            Sc = S[c]
            W = CHI * Sc
            h = Sc // 2
            ps_r = ps[:].rearrange("p (a b) -> p a b", a=CHI)
            if ex == "H":
                # ACT relus the first half-slots; DVE fuses relu of the
                # PSUM second half with the add (one PSUM operand only).
                r1 = scrp.tile([128, CHI, h], bf16, tag=f"r1{Sc}")
                nc.scalar.activation(out=r1[:], in_=ps_r[:, :, 0:h],
                                     func=RELU)
                t1 = scrp.tile([128, CHI, h], bf16, tag=f"t1{Sc}")
                nc.vector._custom_dve(
                    relu2add, out=t1[:], in0=ps_r[:, :, h:Sc],
                    in1=r1[:], accum_out=dve_scrap[:])
            else:
                r = scrp.tile([128, CHI, Sc], bf16, tag=f"r{Sc}")
                if ex == "A":
                    nc.scalar.activation(
                        out=r[:].rearrange("p a b -> p (a b)"),
                        in_=ps[:, 0:W], func=RELU)
                elif ex == "D":
                    nc.vector.tensor_scalar(
                        out=r[:].rearrange("p a b -> p (a b)"),
                        in0=ps[:, 0:W], scalar1=0.0, scalar2=None, op0=MAX)
                else:
                    nc.gpsimd.tensor_scalar(
                        out=r[:].rearrange("p a b -> p (a b)"),
                        in0=ps[:, 0:W], scalar1=0.0, scalar2=None, op0=MAX)
                t1 = scrp.tile([128, CHI, h], bf16, tag=f"t1{Sc}")
                nc.vector.tensor_tensor(out=t1[:], in0=r[:, :, 0:h],
                                        in1=r[:, :, h:Sc], op=ADD)
            h2 = h // 2
            t2 = scrp.tile([128, CHI, h2], bf16, tag=f"t2{Sc}")
            teng = nc.gpsimd if tr == "P" else nc.vector
            teng.tensor_tensor(out=t2[:], in0=t1[:, :, 0:h2],
                               in1=t1[:, :, h2:h], op=ADD)
            reng = nc.gpsimd if tr == "Q" else nc.vector
            reng.tensor_reduce(
                out=msg[:, CHI * c:CHI * (c + 1)], in_=t2[:],
                axis=mybir.AxisListType.X, op=ADD)

        # ---------------- layer 0 ----------------
        def layer0():
            W0 = Wcomb[:, :, 0:D]
            msg = P.tile([D, SH], f32, tag="msg0")
            for c in range(NCH):
                W = CHI * S[c]
                ps = psumC.tile([128, W], f32, tag="chunk")
                for s0 in range(0, W, PIECE):
                    s1 = min(s0 + PIECE, W)
                    nc.tensor.matmul(
                        out=ps[:, s0:s1], lhsT=W0,
                        rhs=peT[:, :, cbase[c] + s0:cbase[c] + s1],
                        start=True, stop=True, perf_mode=DR)
                exit_fold(c, ps, msg, L0_EXIT[c])
            return msg

        def finish0(msg):
            h1T = P.tile([D, SH], f32, tag="h1T")
            msgb = msg[:].bitcast(bf16).rearrange(
                "p (q two) -> p q two", two=2)
            for hh in range(2):
                sl = slice(64 * hh, 64 * (hh + 1))
                ps = psumF.tile([D, 64], f32, tag="fin0")
                nc.tensor.matmul(out=ps[:], lhsT=WnT0, rhs=xs_bf[:, sl],
                                 start=True, stop=False)
                nc.tensor.matmul(out=ps[:], lhsT=WnT0,
                                 rhs=msgb[:, sl, 1:2],
                                 start=False, stop=True)
                if FIN0 == "A":
                    nc.scalar.activation(out=h1T[:, sl], in_=ps[:],
                                         func=RELU, bias=bn0)
                else:
                    nc.vector.tensor_scalar(out=h1T[:, sl], in0=ps[:],
                                            scalar1=bn0, scalar2=0.0,
                                            op0=ADD, op1=MAX)
            return h1T

        # ---------------- gathers (layer 1 node features) ----------------
        def gathers(xgou_src):
            xg1f = P.tile([128, max(Qa, 16)], f32)
            xg1b = P.tile([128, 1, max(Qdp, 128)], bf16)
            plan = []
            for gi, grp in enumerate(AP_GROUPS):
                plan.append(("ap", gi, grp))
            if DG:
                plan.insert(PREP_POS, ("dg",))
            for item in plan:
                if item[0] == "dg":
                    nc.gpsimd.dma_gather(
                        out_ap=xg1b[:, :, 0:Qdp], in_ap=xgou_src,
                        idxs_ap=idx[:, Qa // 16:Qa // 16 + Qdp // 16],
                        num_idxs=Qdp, num_idxs_reg=Qdp, elem_size=D,
                        transpose=True)
                else:
                    _, gi, grp = item
                    lo = apo[grp[0]]
                    hi = apo[grp[-1]] + CHI * S[grp[-1]]
                    nc.gpsimd.ap_gather(
                        out_ap=xg1f[:, lo:hi], in_ap=xe32[:, 0:N],
                        idxs_ap=idx[:, lo // 16:hi // 16],
                        channels=128, num_elems=N, d=1, num_idxs=hi - lo)
            return xg1f, xg1b

        # ---------------- layer 1 ----------------
        def layer1(xg1f, xg1b):
            W1e = Wcomb[0:17, :, D:2 * D]
            xgb = xg1f[:].bitcast(bf16).rearrange(
                "p (q two) -> p q two", two=2)
            msg = P.tile([D, SH], f32, tag="msg1")
            for c in L1_ORDER:
                W = CHI * S[c]
                ps = psumC.tile([128, W], f32, tag="chunk")
                for s0 in range(0, W, PIECE):
                    s1 = min(s0 + PIECE, W)
                    nc.tensor.matmul(
                        out=ps[:, s0:s1], lhsT=W1e,
                        rhs=peT[0:17, :, cbase[c] + s0:cbase[c] + s1],
                        start=True, stop=False, perf_mode=DR)
                    if c in dgo:
                        rhs = xg1b[:, 0, dgo[c] + s0:dgo[c] + s1]
                    else:
                        rhs = xgb[:, apo[c] + s0:apo[c] + s1, 1:2]
                    nc.tensor.matmul(out=ps[:, s0:s1], lhsT=Wn1T, rhs=rhs,
                                     start=False, stop=True)
                exit_fold(c, ps, msg, L1_EXIT[c], TREE1[c])
            return msg

        def finish1(msg, h1T, h2own):
            msgb = msg[:].bitcast(bf16).rearrange(
                "p (q two) -> p q two", two=2)
            h1b = h1T[:].bitcast(bf16).rearrange(
                "p (q two) -> p q two", two=2)
            for hh in FIN1_ORDER:
                sl = slice(64 * hh, 64 * (hh + 1))
                ps = psumF.tile([64, D], f32, tag="fin1")
                nc.tensor.matmul(out=ps[:], lhsT=ones64, rhs=bn1r,
                                 start=True, stop=False)
                nc.tensor.matmul(out=ps[:], lhsT=h1b[:, sl, 1:2],
                                 rhs=WnT1s, start=False, stop=False)
                nc.tensor.matmul(out=ps[:], lhsT=msgb[:, sl, 1:2],
                                 rhs=WnT1, start=False, stop=True)
                if FIN1 == "A":
                    nc.scalar.activation(out=h2own[sl, :], in_=ps[:],
                                         func=RELU)
                else:
                    nc.vector.tensor_scalar(out=h2own[sl, :], in0=ps[:],
                                            scalar1=0.0, scalar2=None,
                                            op0=MAX)
                if OUT != "T":
                    nc.sync.dma_start(out=out_d[sl, :], in_=h2own[sl, :])
            if OUT == "T":
                nc.gpsimd.trigger_dma(count=1, queue_num=0)

        # ---------------- wiring ----------------
        h2own = P.tile([SH, D], f32)

        def out_prep():
            if OUT == "T":
                out_sem = nc.alloc_semaphore("out_dma")
                nc.gpsimd.dma_scatter_add(
                    out_d[:],
                    h2own[:].rearrange("p (g d) -> p g d", g=1),
                    idx[:, IW - SH // 16:IW],
                    SH, SH, D,
                    prepare_only=True, sem=out_sem, queue_num=0)

        if mode == "full":
            msg0 = layer0()
            h1T = finish0(msg0)
            gout = None
            if DG:
                # h1 rows (bf16, [SH, D]) for the dma_gather source
                h1Tb = P.tile([D, SH], bf16)
                nc.scalar.activation(out=h1Tb[:], in_=h1T[:],
                                     func=mybir.ActivationFunctionType.Identity)
                h1r = P.tile([SH, D], bf16)
                nc.sync.dma_start(out=h1r[:], in_=h1Tb[:], transpose=True)
                gin = dramp.tile([SH, D], bf16)
                gout = dramp.tile([N, D], bf16)
                nc.gpsimd.dma_start(out=gin[:], in_=h1r[:])
                nc.gpsimd.collective_compute(
                    "AllGather", mybir.AluOpType.bypass,
                    replica_groups=[list(range(NC))],
                    ins=[gin[:].bitcast(f32).opt()],
                    outs=[gout[:].bitcast(f32).opt()])
            if AP_GROUPS:
                gin2 = dramp.tile([D, SH], f32)
                gout2 = dramp.tile([NC * D, SH], f32)
                nc.gpsimd.dma_start(out=gin2[:], in_=h1T[:])
                nc.gpsimd.collective_compute(
                    "AllGather", mybir.AluOpType.bypass,
                    replica_groups=[list(range(NC))],
                    ins=[gin2[:].opt()], outs=[gout2[:].opt()])
                nc.sync.dma_start(
                    out=xe32[:].rearrange("p (c i) -> p c i", c=NC),
                    in_=gout2[:].rearrange("(c d) i -> d c i", d=D))
            xg1f, xg1b = gathers(gout[:] if gout is not None else None)
            out_prep()
            msg1 = layer1(xg1f, xg1b)
            finish1(msg1, h1T, h2own)
        else:
            # timed variant: no collective; gather source is a host tensor
            xg1f, xg1b = gathers(xgou_d[:] if DG else None)
            out_prep()
            msg0 = layer0()
            h1T = finish0(msg0)
            msg1 = layer1(xg1f, xg1b)
            finish1(msg1, h1T, h2own)

    nc.compile()
    return nc


def _plan(adj):
    """Deal globally degree-sorted rows round-robin to cores (so every
    core's degree profile matches and the SPMD max-over-cores chunk
    padding is tight), bucket into NCH chunks, pad to mult of 4.
    perms[c] holds ABSOLUTE row ids owned by core c, degree-descending."""
    deg = adj.sum(1).astype(np.int64)
    order = np.argsort(-deg, kind="stable")
    perms = [order[c::NC] for c in range(NC)]
    S = []
    for ch in range(NCH):
        mx = max(int(deg[perms[c][CHI * ch:CHI * (ch + 1)]].max())
                 for c in range(NC))
        S.append(max(4, int(-(-mx // 4) * 4)))
    return perms, tuple(S)


def _wrap_idx(L):
    """gather index layout: [128, n//16], idx[p, m] = L[m*16 + p%16]."""
    w = np.asarray(L).reshape(-1, 16).T.astype(np.int16)
    return np.tile(w, (8, 1))


def _host_inputs(inputs):
    """Build the 8 per-core input maps + plan from full inputs."""
    import ml_dtypes

    bf = ml_dtypes.bfloat16
    f8 = ml_dtypes.float8_e4m3
    adj = np.asarray(inputs["adj"], np.float32)
    nodes = np.asarray(inputs["nodes"], np.float32)
    edges = np.asarray(inputs["edges"], np.float32)
    eps = float(np.asarray(inputs["eps"], np.float32).reshape(-1)[0])
    perms, S = _plan(adj)
    Q, cbase, Qa, apo, Qd, Qdp, dgo = _spaces(S)

    # global position of node j in the allgathered (per-core sorted) layout
    gpos = np.empty(N, np.int64)
    for c in range(NC):
        gpos[perms[c]] = c * SH + np.arange(SH)
    sorted_nodes = np.empty((N, D), np.float32)
    for c in range(NC):
        sorted_nodes[c * SH:(c + 1) * SH] = nodes[perms[c]]

    Wne = [np.asarray(inputs["Wne0"], np.float32),
           np.asarray(inputs["Wne1"], np.float32)]
    bne = [np.asarray(inputs["bne0"], np.float32),
           np.asarray(inputs["bne1"], np.float32)]
    # Wcomb: logical contraction rows (interleaved r=2k+t):
    #   0..31 edge rows, 32 bias carrier, 33 mask, 34..161 node rows (l0)
    Wc = np.zeros((2 * KP, 2 * D), np.float32)
    for l in range(2):
        Wc[0:E, l * D:(l + 1) * D] = Wne[l][:, D:D + E].T
        Wc[E, l * D:(l + 1) * D] = bne[l]
        Wc[E + 1, l * D:(l + 1) * D] = MASKW
    Wc[E + 2:E + 2 + D, 0:D] = Wne[0][:, :D].T
    Wcomb = np.clip(Wc, -440, 440).reshape(KP, 2, 2 * D)

    WnB = np.zeros((D, 6 * D + 64), np.float32)
    WnB[:, 0:D] = Wne[1][:, :D].T
    WnB[:, D:2 * D] = np.asarray(inputs["Wn0"], np.float32).T
    WnB[:, 2 * D:3 * D] = np.asarray(inputs["Wn1"], np.float32).T
    WnB[0, 3 * D:3 * D + 64] = 1.0
    WnB[0, 3 * D + 64:4 * D + 64] = np.asarray(inputs["bn1"], np.float32)
    WnB[:, 5 * D + 64:6 * D + 64] = (1.0 + eps) * WnB[:, 2 * D:3 * D]

    bias = np.zeros((D, 4), np.float32)
    bias[:, 0] = np.asarray(inputs["bn0"], np.float32)
    bias[:, 1] = 1.0 + eps

    com = {}

    apch = [c for g in AP_GROUPS for c in g]
    maps = []
    for c in range(NC):
        rows = perms[c]
        pe = np.zeros((Q, 2 * KP), np.float32)
        pe[:, E + 1] = -MASKW                     # mask row: pads -16
        La = np.zeros(Qa, np.int64)
        Ld = np.zeros(Qdp, np.int64)
        for p in range(SH):
            ch = p // CHI
            il = p % CHI
            base = cbase[ch] + il * S[ch]
            nbr = np.nonzero(adj[rows[p]])[0]
            k = len(nbr)
            assert k <= S[ch]
            pe[base:base + k, 0:E] = edges[rows[p], nbr]
            pe[base:base + k, E] = 1.0            # bias carrier
            pe[base:base + k, E + 1] = 0.0        # not padded
            pe[base:base + k, E + 2:E + 2 + D] = nodes[nbr]
            sbase = (dgo[ch] if ch in dgo else apo[ch]) + il * S[ch]
            tgt = Ld if ch in dgo else La
            tgt[sbase:sbase + k] = gpos[nbr]
        m = dict(com)
        WnBc = WnB.copy()
        WnBc[:, 4 * D + 64:5 * D + 64] = (1.0 + eps) * nodes[rows].T
        m["WnB"] = np.ascontiguousarray(WnBc.astype(bf))
        peq = np.clip(pe.T, -440, 440).reshape(KP, 2, Q)
        m["peT_sh"] = np.ascontiguousarray(
            np.concatenate([Wcomb, peq], axis=2).astype(f8))
        m["xb_sh"] = np.ascontiguousarray(
            np.concatenate([nodes[rows].T, bias], axis=1))
        m["idx_sh"] = np.ascontiguousarray(
            np.concatenate([_wrap_idx(La), _wrap_idx(Ld),
                            _wrap_idx(np.arange(SH))], axis=1))
        m["xe32_sh"] = np.ascontiguousarray(sorted_nodes.T)
        m["xgou_sh"] = np.ascontiguousarray(sorted_nodes.astype(bf))
        maps.append(m)
    return maps, perms, S


def _get_runner(S):
    """Build (once per S) a cached jit(shard_map) callable."""
    key = ("runner", S)
    if key in _cache:
        return _cache[key]
    import jax
    from jax.sharding import Mesh, PartitionSpec, NamedSharding
    from jax.experimental.shard_map import shard_map
    import concourse.mybir as mybir
    from concourse import bass2jax
    from concourse.bass2jax import _bass_exec_p, partition_id_tensor

    nckey = ("nc", S)
    if nckey not in _cache:
        _cache[nckey] = _build_nc("full", S)
    nc = _cache[nckey]
    bass2jax.install_neuronx_cc_hook()

    in_names, out_names, out_avals, zero_outs = [], [], [], []
    partition_name = nc.partition_id_tensor.name if nc.partition_id_tensor else None
    for alloc in nc.m.functions[0].allocations:
        if not isinstance(alloc, mybir.MemoryLocationSet):
            continue
        name = alloc.memorylocations[0].name
        if alloc.kind == "ExternalInput":
            if name != partition_name:
                in_names.append(name)
        elif alloc.kind == "ExternalOutput":
            shape = list(alloc.tensor_shape)
            dtype = np.dtype(mybir.dt.np(alloc.dtype))
            out_avals.append(jax.core.ShapedArray(shape, dtype))
            out_names.append(name)
            zero_outs.append(np.zeros(shape, dtype))

    n_params = len(in_names)
    all_in_names = list(in_names) + list(out_names)
    if partition_name is not None:
        all_in_names.append(partition_name)

    def _body(*args):
        operands = list(args)
        if partition_name is not None:
            operands.append(partition_id_tensor())
        outs = _bass_exec_p.bind(
            *operands,
            out_avals=tuple(out_avals),
            in_names=tuple(all_in_names),
            out_names=tuple(out_names),
            lowering_input_output_aliases=(),
            sim_require_finite=True,
            sim_require_nnan=True,
            nc=nc,
        )
        return tuple(outs)

    devices = jax.devices()[:NC]
    mesh = Mesh(np.asarray(devices), ("core",))
    n_outs = len(out_names)
    fn = jax.jit(
        shard_map(_body, mesh=mesh,
                  in_specs=(PartitionSpec("core"),) * (n_params + n_outs),
                  out_specs=(PartitionSpec("core"),) * n_outs,
                  check_rep=False),
        keep_unused=True)
    sh = NamedSharding(mesh, PartitionSpec("core"))
    dev_zeros = [
        jax.device_put(np.zeros((NC * z.shape[0], *z.shape[1:]), z.dtype), sh)
        for z in zero_outs
    ]

    def run(maps):
        dev_in = []
        for nm in in_names:
            arrs = [
                jax.device_put(np.asarray(maps[c][nm]), devices[c])
                for c in range(NC)
            ]
            shp = arrs[0].shape
            glob = jax.make_array_from_single_device_arrays(
                (NC * shp[0], *shp[1:]), sh, arrs)
            dev_in.append(glob)
        outs = fn(*dev_in, *dev_zeros)
        oi = out_names.index("out")
        return np.asarray(outs[oi]).reshape(NC, SH, D)

    _cache[key] = run
    return run


def kernel(**inputs):
    maps, perms, S = _host_inputs(inputs)
    run = _get_runner(S)
    raw = run(maps)                                # [NC, SH, D], sorted rows
    out = np.empty((N, D), np.float32)
    for c in range(NC):
        out[perms[c]] = raw[c]
    return np.ascontiguousarray(out.astype(np.float32))


if __name__ == "__main__":
    _build_nc("nocc")
    print("build+compile OK")


# revision 55
# speedup vs baseline: 1.2937x; 1.0016x over previous
"""DGINConv (2-layer GIN with edge features) Trainium2 kernel — fp8 DoubleRow.

Math (per layer, reference):
    ne[i,j,:] = relu(Wnode@x[j] + We@edges[i,j,:] + bne)
    msg[i,:]  = sum_{j: adj[i,j]=1} ne[i,j,:]
    h[i,:]    = relu(Wn @ ((1+eps)*x[i] + msg[i]) + bn)

adj density ~3%: each own row's ~31 neighbors are packed into padded slots
(host-side): own rows degree-sorted, grouped into 8 chunks of 16 rows,
chunk c padded to S_c slots/row (mult of 4). Q = 16*sum(S_c) per core.

Key structure (vs the previous bf16/identity-inject version):
- Layer 0: node term + edge term + bias + pad-mask fused into ONE fp8
  DoubleRow matmul per 256-col piece: rhs peT [81, 2, W] (logical
  contraction row r = 2k+t: 32 edge rows ++ bias carrier ++ mask(-16) ++
  128 host-gathered x rows, all fp8e4m3), lhsT Wcomb [81, 2, 128] fp8.
  DoubleRow streams 2 fp8 rows/cycle -> 0.5 PE cycles per slot (4x less
  PE time than two bf16 matmuls). Pads produce psum=-256 -> relu kills
  them; no -1e9 plumbing.
- Layer 1 reuses peT partitions 0:17 (the row interleave puts logical
  rows 0..33 = edges+bias+mask exactly there) as a [17, 2, W] fp8
  DoubleRow matmul with layer-1 weights; the node term is ap_gather'd
  per slot from an f32 copy of the allgathered features ([128, N] ->
  [128, Q]) and added via one bf16 K=128 matmul into the same PSUM
  (rhs = high-half bitcast of the gathered f32). No hbT pass, no
  identity-inject matmul, no transposes. (SWDGE dma_gather/scatter are
  NOT used: unsupported by this execution backend.)
- Exits per chunk (knob): 'A' = ACT relu [128, W] -> bf16 + DVE pairwise
  tree; 'H' = ACT relus the first half-slots, then a custom DVE op
  relu(in0)+in1 fuses the second half's relu (its single PSUM operand)
  with tree level 1. t2 tree level on DVE or Pool (TREE1), reduce DVE.
- finishes exploit linearity: Wn((1+eps)x + msg) = Wn(1+eps)x + Wn msg,
  so no z staging pass - two accumulating matmuls per half against a
  host-scaled bf16 x copy and a high-half-bitcast bf16 view of the f32
  msg tile. finish l0 keeps [D, rows] (ACT relu + bn0 bias); finish l1
  is emitted flipped (stationary = h1/msg columns, moving = WnT1, bn1
  preloaded into PSUM by a K=1 matmul) so the output lands [rows, D]
  f32 and DMAs straight out.
- Weights ride in the leading 256 columns of the peT tensor (one DMA);
  bias columns ride in the xown tensor. All input DMAs issue from the
  SP queue (DMACopy holds the issuing engine's sequencer, and HWDGE is
  a serial ~630ns/DMA resource - keep it off compute engines).

Distribution: destination rows sharded 8 ways; nodes/weights replicated;
updated node features exchanged between layers via AllGather (full mode;
an f32 [D, SH] allgather feeds the gather source via one rearrange DMA).
The timed 'nocc' variant is the same program with host-fed stand-ins for
the collective outputs. Final output rows are un-permuted on the host.
"""

import sys

if "/opt/trn_rl_repo" not in sys.path:
    sys.path.insert(0, "/opt/trn_rl_repo")

import numpy as np

N, D, E, NC = 1024, 128, 32, 8
SH = N // NC          # 128 rows per core
NCH = 8               # chunks of sorted own-rows
CHI = SH // NCH       # 16 rows per chunk
KP = 81               # DoubleRow partitions: 2*81 = 162 >= 32+2+128
MASKW = 16.0          # mask weight; mask rhs = -16 -> psum -256 on pads

S_DEFAULT = (52, 40, 36, 32, 32, 32, 28, 28)

# ---------------- tuning knobs (overridable via set_cfg) ----------------
DG = ()                       # dma_gather unsupported on this backend
AP_GROUPS = ((0, 1), (2, 3, 4), (5, 6, 7))    # ap_gather call groups
PREP_POS = 0                  # index in pool sequence where dma_gather goes
L0_EXIT = "HAHAHAHA"          # A=ACT relu | D=DVE relu | H=ACT half + fused DVE
L1_EXIT = "HAAHAHHA"          # P=Pool relu (layer1 only)
TREE1 = "DDPPPPPP"            # layer1 t2 engine: D=DVE, P=Pool
L1_ORDER = (0, 1, 2, 3, 4, 5, 6, 7)   # layer1 chunk processing order
FIN0 = "A"                    # layer0 finish relu engine: A=ACT, D=DVE
FIN1 = "DD"                   # layer1 finish relu engine per half
FIN1_ORDER = (0, 1)           # finish1 half emission order
OUT = "S"                     # output: T=SWDGE prep+trigger scatter, S=dma
PET_SPLITS = (3, 4)           # peT piece boundaries (cbase indices)
PIECE = 256                   # DoubleRow piece width (<=256)
# DMA issue plan: queue S=nc.sync, C=nc.scalar, V=nc.vector; names below
DMA_PLAN = (
    ("S", "peT0"), ("S", "idx"), ("S", "xe32"),
    ("S", "peT1"), ("S", "peT2"), ("S", "WnB"), ("S", "xb"),
)

_KNOBS = ("DG", "AP_GROUPS", "PREP_POS", "L0_EXIT", "L1_EXIT", "TREE1",
          "L1_ORDER", "FIN0", "FIN1", "FIN1_ORDER", "OUT", "PIECE", "DMA_PLAN",
          "PET_SPLITS")


def set_cfg(**kw):
    g = globals()
    for k, v in kw.items():
        assert k in _KNOBS, k
        g[k] = v

_cache = {}
_CUSTOM = {}


def _ensure_relu2add():
    """Register RELUADD1_GIN: out = relu(in0) + in1; accum = sum.

    in0 may be PSUM (the un-relu'd second half of a chunk); in1 is the
    already-relu'd first half in SBUF — only one PSUM operand, which is
    all the ISA allows.
    """
    if "op" in _CUSTOM:
        return _CUSTOM["op"]
    import concourse.dve_ops as dve_ops
    from concourse.dve_spec import Spec, Src0, Src1, relu, lower, _has_src1
    from concourse.dve_spec import Zero
    from concourse.dve_uop import DveOpSpec
    from operator import add

    name = "RELUADD1_GIN"

    def _ref(in0, in1, c0, c1, c2):
        b = (dve_ops._dve_relu(in0.astype(np.float32))
             + in1.astype(np.float32).reshape(in0.shape))
        return b, b.reshape(b.shape[0], -1).sum(axis=-1, keepdims=True)

    spec = Spec(body=relu(Src0) + Src1, accum=add, accum_init=Zero,
                reference=_ref)
    row = dve_ops._CUSTOM_DVE_ROW_BASE + len(dve_ops.OPS)
    assert row < 0x20
    shas = {}
    for ver in ("v3", "v4"):
        try:
            s = DveOpSpec(name=name, opcode=row, uops=lower(spec, ver=ver),
                          rd1_en=_has_src1(spec))
            shas[ver] = s.sha(ver)
        except Exception:
            pass
    op = dve_ops.DveOp(name, spec, subdim=False, uops_sha=shas)
    dve_ops.OPS.append(op)
    dve_ops.CUSTOM_DVE_SPECS[name] = spec
    dve_ops._SUB_OPCODE_FOR_NAME[name] = row
    _CUSTOM["op"] = op
    return op


def _spaces(S):
    """Derive slot-space geometry from chunk sizes."""
    S = tuple(S)
    Q = CHI * sum(S)
    cbase = [CHI * sum(S[:c]) for c in range(NCH)]
    apch = [c for g in AP_GROUPS for c in g]
    Qa = CHI * sum(S[c] for c in apch)
    apo = {}
    off = 0
    for c in apch:
        apo[c] = off
        off += CHI * S[c]
    Qd = CHI * sum(S[c] for c in DG)
    Qdp = -(-Qd // 128) * 128
    dgo = {}
    off = 0
    for c in DG:
        dgo[c] = off
        off += CHI * S[c]
    return Q, cbase, Qa, apo, Qd, Qdp, dgo


def _build_nc(mode="full", S=S_DEFAULT):
    from contextlib import ExitStack

    import concourse.mybir as mybir
    import concourse.tile as tile
    from concourse import bacc

    f32 = mybir.dt.float32
    bf16 = mybir.dt.bfloat16
    fp8 = mybir.dt.float8e4
    i16 = mybir.dt.int16
    RELU = mybir.ActivationFunctionType.Relu
    ADD = mybir.AluOpType.add
    MAX = mybir.AluOpType.max
    MULT = mybir.AluOpType.mult
    DR = mybir.MatmulPerfMode.DoubleRow

    relu2add = _ensure_relu2add()
    S = tuple(S)
    Q, cbase, Qa, apo, Qd, Qdp, dgo = _spaces(S)
    IW = Qa // 16 + Qdp // 16 + SH // 16

    nc = bacc.Bacc("TRN2", target_bir_lowering=False, debug=False,
                   enable_asserts=False, num_devices=NC)

    def din(name, shape, dt=None):
        return nc.dram_tensor(name, shape, dt or f32, kind="ExternalInput").ap()

    WC = 2 * D   # leading peT cols hold Wcomb (shared weights)
    peT_d = din("peT_sh", [KP, 2, WC + Q], fp8)
    WnB_d = din("WnB", [D, 6 * D + 64], bf16)  # Wn1T|WnT0|WnT1|row0:ones,bn1|xs|WnT1s
    xb_d = din("xb_sh", [D, SH + 4])           # xown ++ (bn0 | 1+eps)
    idx_d = din("idx_sh", [128, IW], i16)
    if mode != "full":
        xe32_d = din("xe32_sh", [D, N])        # f32 allgathered-x stand-in
        xgou_d = din("xgou_sh", [N, D], bf16)  # node-major bf16 stand-in
    out_d = nc.dram_tensor("out", [SH, D], f32, kind="ExternalOutput").ap()

    with tile.TileContext(nc) as tc, ExitStack() as ctx:
        P = ctx.enter_context(tc.tile_pool(name="persist", bufs=1))
        dramp = ctx.enter_context(tc.tile_pool(name="dram", bufs=1, space="DRAM"))
        psumC = ctx.enter_context(tc.tile_pool(name="psumC", bufs=3, space="PSUM"))
        psumF = ctx.enter_context(tc.tile_pool(name="psumF", bufs=1, space="PSUM"))
        scrp = ctx.enter_context(tc.tile_pool(name="scr", bufs=3))

        # ---------------- input DMAs ----------------
        peTW = P.tile([KP, 2, WC + Q], fp8)
        peT = peTW[:, :, WC:]
        Wcomb = peTW[:, :, 0:WC]
        WnB = P.tile([D, 6 * D + 64], bf16)
        xb = P.tile([D, SH + 4], f32)
        xown = xb[:, 0:SH]
        bias = xb[:, SH:SH + 4]
        idx = P.tile([128, IW], i16)
        xe32 = P.tile([D, N], f32)

        qmap = {"S": nc.sync, "C": nc.scalar, "V": nc.vector}
        pbounds = ([0] + [WC + cbase[i] for i in PET_SPLITS]
                   + [WC + Q])

        def issue(q, name):
            eng = qmap[q]
            if name.startswith("peT"):
                i = int(name[3])
                lo, hi = pbounds[i], pbounds[i + 1]
                eng.dma_start(out=peTW[:, :, lo:hi], in_=peT_d[:, :, lo:hi])
            elif name == "xe32":
                if mode != "full":
                    eng.dma_start(out=xe32[:], in_=xe32_d[:])
            elif name == "WnB":
                eng.dma_start(out=WnB[:], in_=WnB_d[:])
            elif name == "xb":
                eng.dma_start(out=xb[:], in_=xb_d[:])
            elif name == "idx":
                eng.dma_start(out=idx[:], in_=idx_d[:])

        for q, name in DMA_PLAN:
            issue(q, name)

        dve_scrap = P.tile([128, 1], f32)
        Wn1T = WnB[:, 0:D]
        WnT0 = WnB[:, D:2 * D]
        WnT1 = WnB[:, 2 * D:3 * D]
        ones64 = WnB[0:1, 3 * D:3 * D + 64]
        bn1r = WnB[0:1, 3 * D + 64:4 * D + 64]
        xs_bf = WnB[:, 4 * D + 64:5 * D + 64]   # (1+eps)*xown bf16
        WnT1s = WnB[:, 5 * D + 64:6 * D + 64]   # (1+eps)*Wn1.T bf16
        bn0 = bias[:, 0:1]
        opse = bias[:, 1:2]

        # ---------------- exit + fold ----------------
        def exit_fold(c, ps, msg, ex, tr="D"):
            """PSUM [128, CHI*S[c]] -> relu -> segment sum -> msg cols."""
            Sc = S[c]
            W = CHI * Sc
            h = Sc // 2
            ps_r = ps[:].rearrange("p (a b) -> p a b", a=CHI)
            if ex == "H":
                # ACT relus the first half-slots; DVE fuses relu of the
                # PSUM second half with the add (one PSUM operand only).
                r1 = scrp.tile([128, CHI, h], bf16, tag=f"r1{Sc}")
                nc.scalar.activation(out=r1[:], in_=ps_r[:, :, 0:h],
                                     func=RELU)
                t1 = scrp.tile([128, CHI, h], bf16, tag=f"t1{Sc}")
                nc.vector._custom_dve(
                    relu2add, out=t1[:], in0=ps_r[:, :, h:Sc],
                    in1=r1[:], accum_out=dve_scrap[:])
            else:
                r = scrp.tile([128, CHI, Sc], bf16, tag=f"r{Sc}")
                if ex == "A":
                    nc.scalar.activation(
                        out=r[:].rearrange("p a b -> p (a b)"),
                        in_=ps[:, 0:W], func=RELU)
                elif ex == "D":
                    nc.vector.tensor_scalar(
                        out=r[:].rearrange("p a b -> p (a b)"),
                        in0=ps[:, 0:W], scalar1=0.0, scalar2=None, op0=MAX)
                else:
                    nc.gpsimd.tensor_scalar(
                        out=r[:].rearrange("p a b -> p (a b)"),
                        in0=ps[:, 0:W], scalar1=0.0, scalar2=None, op0=MAX)
                t1 = scrp.tile([128, CHI, h], bf16, tag=f"t1{Sc}")
                nc.vector.tensor_tensor(out=t1[:], in0=r[:, :, 0:h],
                                        in1=r[:, :, h:Sc], op=ADD)
            h2 = h // 2
            t2 = scrp.tile([128, CHI, h2], bf16, tag=f"t2{Sc}")
            teng = nc.gpsimd if tr == "P" else nc.vector
            teng.tensor_tensor(out=t2[:], in0=t1[:, :, 0:h2],
                               in1=t1[:, :, h2:h], op=ADD)
            reng = nc.gpsimd if tr == "Q" else nc.vector
            reng.tensor_reduce(
                out=msg[:, CHI * c:CHI * (c + 1)], in_=t2[:],
                axis=mybir.AxisListType.X, op=ADD)

        # ---------------- layer 0 ----------------
        def layer0():
            W0 = Wcomb[:, :, 0:D]
            msg = P.tile([D, SH], f32, tag="msg0")
            for c in range(NCH):
                W = CHI * S[c]
                ps = psumC.tile([128, W], f32, tag="chunk")
                for s0 in range(0, W, PIECE):
                    s1 = min(s0 + PIECE, W)
                    nc.tensor.matmul(
                        out=ps[:, s0:s1], lhsT=W0,
                        rhs=peT[:, :, cbase[c] + s0:cbase[c] + s1],
                        start=True, stop=True, perf_mode=DR)
                exit_fold(c, ps, msg, L0_EXIT[c])
            return msg

        def finish0(msg):
            h1T = P.tile([D, SH], f32, tag="h1T")
            msgb = msg[:].bitcast(bf16).rearrange(
                "p (q two) -> p q two", two=2)
            for hh in range(2):
                sl = slice(64 * hh, 64 * (hh + 1))
                ps = psumF.tile([D, 64], f32, tag="fin0")
                nc.tensor.matmul(out=ps[:], lhsT=WnT0, rhs=xs_bf[:, sl],
                                 start=True, stop=False)
                nc.tensor.matmul(out=ps[:], lhsT=WnT0,
                                 rhs=msgb[:, sl, 1:2],
                                 start=False, stop=True)
                if FIN0 == "A":
                    nc.scalar.activation(out=h1T[:, sl], in_=ps[:],
                                         func=RELU, bias=bn0)
                else:
                    nc.vector.tensor_scalar(out=h1T[:, sl], in0=ps[:],
                                            scalar1=bn0, scalar2=0.0,
                                            op0=ADD, op1=MAX)
            return h1T

        # ---------------- gathers (layer 1 node features) ----------------
        def gathers(xgou_src):
            xg1f = P.tile([128, max(Qa, 16)], f32)
            xg1b = P.tile([128, 1, max(Qdp, 128)], bf16)
            plan = []
            for gi, grp in enumerate(AP_GROUPS):
                plan.append(("ap", gi, grp))
            if DG:
                plan.insert(PREP_POS, ("dg",))
            for item in plan:
                if item[0] == "dg":
                    nc.gpsimd.dma_gather(
                        out_ap=xg1b[:, :, 0:Qdp], in_ap=xgou_src,
                        idxs_ap=idx[:, Qa // 16:Qa // 16 + Qdp // 16],
                        num_idxs=Qdp, num_idxs_reg=Qdp, elem_size=D,
                        transpose=True)
                else:
                    _, gi, grp = item
                    lo = apo[grp[0]]
                    hi = apo[grp[-1]] + CHI * S[grp[-1]]
                    nc.gpsimd.ap_gather(
                        out_ap=xg1f[:, lo:hi], in_ap=xe32[:, 0:N],
                        idxs_ap=idx[:, lo // 16:hi // 16],
                        channels=128, num_elems=N, d=1, num_idxs=hi - lo)
            return xg1f, xg1b

        # ---------------- layer 1 ----------------
        def layer1(xg1f, xg1b):
            W1e = Wcomb[0:17, :, D:2 * D]
            xgb = xg1f[:].bitcast(bf16).rearrange(
                "p (q two) -> p q two", two=2)
            msg = P.tile([D, SH], f32, tag="msg1")
            for c in L1_ORDER:
                W = CHI * S[c]
                ps = psumC.tile([128, W], f32, tag="chunk")
                for s0 in range(0, W, PIECE):
                    s1 = min(s0 + PIECE, W)
                    nc.tensor.matmul(
                        out=ps[:, s0:s1], lhsT=W1e,
                        rhs=peT[0:17, :, cbase[c] + s0:cbase[c] + s1],
                        start=True, stop=False, perf_mode=DR)
                    if c in dgo:
                        rhs = xg1b[:, 0, dgo[c] + s0:dgo[c] + s1]
                    else:
                        rhs = xgb[:, apo[c] + s0:apo[c] + s1, 1:2]
                    nc.tensor.matmul(out=ps[:, s0:s1], lhsT=Wn1T, rhs=rhs,
                                     start=False, stop=True)
                exit_fold(c, ps, msg, L1_EXIT[c], TREE1[c])
            return msg

        def finish1(msg, h1T, h2own):
            msgb = msg[:].bitcast(bf16).rearrange(
                "p (q two) -> p q two", two=2)
            h1b = h1T[:].bitcast(bf16).rearrange(
                "p (q two) -> p q two", two=2)
            for hh in FIN1_ORDER:
                sl = slice(64 * hh, 64 * (hh + 1))
                ps = psumF.tile([64, D], f32, tag="fin1")
                nc.tensor.matmul(out=ps[:], lhsT=ones64, rhs=bn1r,
                                 start=True, stop=False)
                nc.tensor.matmul(out=ps[:], lhsT=h1b[:, sl, 1:2],
                                 rhs=WnT1s, start=False, stop=False)
                nc.tensor.matmul(out=ps[:], lhsT=msgb[:, sl, 1:2],
                                 rhs=WnT1, start=False, stop=True)
                if FIN1 == "A":
                    nc.scalar.activation(out=h2own[sl, :], in_=ps[:],
                                         func=RELU)
                else:
                    nc.vector.tensor_scalar(out=h2own[sl, :], in0=ps[:],
                                            scalar1=0.0, scalar2=None,
                                            op0=MAX)
                if OUT != "T":
                    nc.sync.dma_start(out=out_d[sl, :], in_=h2own[sl, :])
            if OUT == "T":
                nc.gpsimd.trigger_dma(count=1, queue_num=0)

        # ---------------- wiring ----------------
        h2own = P.tile([SH, D], f32)

        def out_prep():
            if OUT == "T":
                out_sem = nc.alloc_semaphore("out_dma")
                nc.gpsimd.dma_scatter_add(
                    out_d[:],
                    h2own[:].rearrange("p (g d) -> p g d", g=1),
                    idx[:, IW - SH // 16:IW],
                    SH, SH, D,
                    prepare_only=True, sem=out_sem, queue_num=0)

        if mode == "full":
            msg0 = layer0()
            h1T = finish0(msg0)
            gout = None
            if DG:
                # h1 rows (bf16, [SH, D]) for the dma_gather source
                h1Tb = P.tile([D, SH], bf16)
                nc.scalar.activation(out=h1Tb[:], in_=h1T[:],
                                     func=mybir.ActivationFunctionType.Identity)
                h1r = P.tile([SH, D], bf16)
                nc.sync.dma_start(out=h1r[:], in_=h1Tb[:], transpose=True)
                gin = dramp.tile([SH, D], bf16)
                gout = dramp.tile([N, D], bf16)
                nc.gpsimd.dma_start(out=gin[:], in_=h1r[:])
                nc.gpsimd.collective_compute(
                    "AllGather", mybir.AluOpType.bypass,
                    replica_groups=[list(range(NC))],
                    ins=[gin[:].bitcast(f32).opt()],
                    outs=[gout[:].bitcast(f32).opt()])
            if AP_GROUPS:
                gin2 = dramp.tile([D, SH], f32)
                gout2 = dramp.tile([NC * D, SH], f32)
                nc.gpsimd.dma_start(out=gin2[:], in_=h1T[:])
                nc.gpsimd.collective_compute(
                    "AllGather", mybir.AluOpType.bypass,
                    replica_groups=[list(range(NC))],
                    ins=[gin2[:].opt()], outs=[gout2[:].opt()])
                nc.sync.dma_start(
                    out=xe32[:].rearrange("p (c i) -> p c i", c=NC),
                    in_=gout2[:].rearrange("(c d) i -> d c i", d=D))
            xg1f, xg1b = gathers(gout[:] if gout is not None else None)
            out_prep()
            msg1 = layer1(xg1f, xg1b)
            finish1(msg1, h1T, h2own)
        else:
            # timed variant: no collective; gather source is a host tensor
            xg1f, xg1b = gathers(xgou_d[:] if DG else None)
            out_prep()
            msg0 = layer0()
            h1T = finish0(msg0)
            msg1 = layer1(xg1f, xg1b)
            finish1(msg1, h1T, h2own)

    nc.compile()
    return nc


def _plan(adj):
    """Deal globally degree-sorted rows round-robin to cores (so every
    core's degree profile matches and the SPMD max-over-cores chunk
    padding is tight), bucket into NCH chunks, pad to mult of 4.
    perms[c] holds ABSOLUTE row ids owned by core c, degree-descending."""
    deg = adj.sum(1).astype(np.int64)
    order = np.argsort(-deg, kind="stable")
    perms = [order[c::NC] for c in range(NC)]
    S = []
    for ch in range(NCH):
        mx = max(int(deg[perms[c][CHI * ch:CHI * (ch + 1)]].max())
                 for c in range(NC))
        S.append(max(4, int(-(-mx // 4) * 4)))
    return perms, tuple(S)


def _wrap_idx(L):
    """gather index layout: [128, n//16], idx[p, m] = L[m*16 + p%16]."""
    w = np.asarray(L).reshape(-1, 16).T.astype(np.int16)
    return np.tile(w, (8, 1))


def _host_inputs(inputs):
    """Build the 8 per-core input maps + plan from full inputs."""
    import ml_dtypes

    bf = ml_dtypes.bfloat16
    f8 = ml_dtypes.float8_e4m3
    adj = np.asarray(inputs["adj"], np.float32)
    nodes = np.asarray(inputs["nodes"], np.float32)
    edges = np.asarray(inputs["edges"], np.float32)
    eps = float(np.asarray(inputs["eps"], np.float32).reshape(-1)[0])
    perms, S = _plan(adj)
    Q, cbase, Qa, apo, Qd, Qdp, dgo = _spaces(S)

    # global position of node j in the allgathered (per-core sorted) layout
    gpos = np.empty(N, np.int64)
    for c in range(NC):
        gpos[perms[c]] = c * SH + np.arange(SH)
    sorted_nodes = np.empty((N, D), np.float32)
    for c in range(NC):
        sorted_nodes[c * SH:(c + 1) * SH] = nodes[perms[c]]

    Wne = [np.asarray(inputs["Wne0"], np.float32),
           np.asarray(inputs["Wne1"], np.float32)]
    bne = [np.asarray(inputs["bne0"], np.float32),
           np.asarray(inputs["bne1"], np.float32)]
    # Wcomb: logical contraction rows (interleaved r=2k+t):
    #   0..31 edge rows, 32 bias carrier, 33 mask, 34..161 node rows (l0)
    Wc = np.zeros((2 * KP, 2 * D), np.float32)
    for l in range(2):
        Wc[0:E, l * D:(l + 1) * D] = Wne[l][:, D:D + E].T
        Wc[E, l * D:(l + 1) * D] = bne[l]
        Wc[E + 1, l * D:(l + 1) * D] = MASKW
    Wc[E + 2:E + 2 + D, 0:D] = Wne[0][:, :D].T
    Wcomb = np.clip(Wc, -440, 440).reshape(KP, 2, 2 * D)

    WnB = np.zeros((D, 6 * D + 64), np.float32)
    WnB[:, 0:D] = Wne[1][:, :D].T
    WnB[:, D:2 * D] = np.asarray(inputs["Wn0"], np.float32).T
    WnB[:, 2 * D:3 * D] = np.asarray(inputs["Wn1"], np.float32).T
    WnB[0, 3 * D:3 * D + 64] = 1.0
    WnB[0, 3 * D + 64:4 * D + 64] = np.asarray(inputs["bn1"], np.float32)
    WnB[:, 5 * D + 64:6 * D + 64] = (1.0 + eps) * WnB[:, 2 * D:3 * D]

    bias = np.zeros((D, 4), np.float32)
    bias[:, 0] = np.asarray(inputs["bn0"], np.float32)
    bias[:, 1] = 1.0 + eps

    com = {}

    apch = [c for g in AP_GROUPS for c in g]
    maps = []
    for c in range(NC):
        rows = perms[c]
        pe = np.zeros((Q, 2 * KP), np.float32)
        pe[:, E + 1] = -MASKW                     # mask row: pads -16
        La = np.zeros(Qa, np.int64)
        Ld = np.zeros(Qdp, np.int64)
        for p in range(SH):
            ch = p // CHI
            il = p % CHI
            base = cbase[ch] + il * S[ch]
            nbr = np.nonzero(adj[rows[p]])[0]
            k = len(nbr)
            assert k <= S[ch]
            pe[base:base + k, 0:E] = edges[rows[p], nbr]
            pe[base:base + k, E] = 1.0            # bias carrier
            pe[base:base + k, E + 1] = 0.0        # not padded
            pe[base:base + k, E + 2:E + 2 + D] = nodes[nbr]
            sbase = (dgo[ch] if ch in dgo else apo[ch]) + il * S[ch]
            tgt = Ld if ch in dgo else La
            tgt[sbase:sbase + k] = gpos[nbr]
        m = dict(com)
        WnBc = WnB.copy()
        WnBc[:, 4 * D + 64:5 * D + 64] = (1.0 + eps) * nodes[rows].T
        m["WnB"] = np.ascontiguousarray(WnBc.astype(bf))
        peq = np.clip(pe.T, -440, 440).reshape(KP, 2, Q)
        m["peT_sh"] = np.ascontiguousarray(
            np.concatenate([Wcomb, peq], axis=2).astype(f8))
        m["xb_sh"] = np.ascontiguousarray(
            np.concatenate([nodes[rows].T, bias], axis=1))
        m["idx_sh"] = np.ascontiguousarray(
            np.concatenate([_wrap_idx(La), _wrap_idx(Ld),
                            _wrap_idx(np.arange(SH))], axis=1))
        m["xe32_sh"] = np.ascontiguousarray(sorted_nodes.T)
        m["xgou_sh"] = np.ascontiguousarray(sorted_nodes.astype(bf))
        maps.append(m)
    return maps, perms, S


def _get_runner(S):
    """Build (once per S) a cached jit(shard_map) callable."""
    key = ("runner", S)
    if key in _cache:
        return _cache[key]
    import jax
    from jax.sharding import Mesh, PartitionSpec, NamedSharding
    from jax.experimental.shard_map import shard_map
    import concourse.mybir as mybir
    from concourse import bass2jax
    from concourse.bass2jax import _bass_exec_p, partition_id_tensor

    nckey = ("nc", S)
    if nckey not in _cache:
        _cache[nckey] = _build_nc("full", S)
    nc = _cache[nckey]
    bass2jax.install_neuronx_cc_hook()

    in_names, out_names, out_avals, zero_outs = [], [], [], []
    partition_name = nc.partition_id_tensor.name if nc.partition_id_tensor else None
    for alloc in nc.m.functions[0].allocations:
        if not isinstance(alloc, mybir.MemoryLocationSet):
            continue
        name = alloc.memorylocations[0].name
        if alloc.kind == "ExternalInput":
            if name != partition_name:
                in_names.append(name)
        elif alloc.kind == "ExternalOutput":
            shape = list(alloc.tensor_shape)
            dtype = np.dtype(mybir.dt.np(alloc.dtype))
            out_avals.append(jax.core.ShapedArray(shape, dtype))
            out_names.append(name)
            zero_outs.append(np.zeros(shape, dtype))

    n_params = len(in_names)
    all_in_names = list(in_names) + list(out_names)
    if partition_name is not None:
        all_in_names.append(partition_name)

    def _body(*args):
        operands = list(args)
        if partition_name is not None:
            operands.append(partition_id_tensor())
        outs = _bass_exec_p.bind(
            *operands,
            out_avals=tuple(out_avals),
            in_names=tuple(all_in_names),
            out_names=tuple(out_names),
            lowering_input_output_aliases=(),
            sim_require_finite=True,
            sim_require_nnan=True,
            nc=nc,
        )
        return tuple(outs)

    devices = jax.devices()[:NC]
    mesh = Mesh(np.asarray(devices), ("core",))
    n_outs = len(out_names)
    fn = jax.jit(
        shard_map(_body, mesh=mesh,
                  in_specs=(PartitionSpec("core"),) * (n_params + n_outs),
                  out_specs=(PartitionSpec("core"),) * n_outs,
                  check_rep=False),
        keep_unused=True)
    sh = NamedSharding(mesh, PartitionSpec("core"))
    dev_zeros = [
        jax.device_put(np.zeros((NC * z.shape[0], *z.shape[1:]), z.dtype), sh)
        for z in zero_outs
    ]

    def run(maps):
        dev_in = []
        for nm in in_names:
            arrs = [
                jax.device_put(np.asarray(maps[c][nm]), devices[c])
                for c in range(NC)
            ]
            shp = arrs[0].shape
            glob = jax.make_array_from_single_device_arrays(
                (NC * shp[0], *shp[1:]), sh, arrs)
            dev_in.append(glob)
        outs = fn(*dev_in, *dev_zeros)
        oi = out_names.index("out")
        return np.asarray(outs[oi]).reshape(NC, SH, D)

    _cache[key] = run
    return run


def kernel(**inputs):
    maps, perms, S = _host_inputs(inputs)
    run = _get_runner(S)
    raw = run(maps)                                # [NC, SH, D], sorted rows
    out = np.empty((N, D), np.float32)
    for c in range(NC):
        out[perms[c]] = raw[c]
    return np.ascontiguousarray(out.astype(np.float32))


if __name__ == "__main__":
    _build_nc("nocc")
    print("build+compile OK")
